# revision 1
# baseline (speedup 1.0000x reference)
"""Trainium2 Bass kernel for the GNN k-hop subgraph encoder (GIN, L=2, D=256).

Strategy (8 cores, graph-parallel):
  - Host: sort subgraph nodes by indicator (center id); shard at center
    boundaries (2500 centers/core); slotted per-core layout so every
    128-center block owns a fixed number of 128-row tiles.
  - Layer 1 needs NO gather: node/edge embedding types have tiny
    cardinality (atom/chir/bond in {0,1,2}), so the layer-1 aggregation is
    counts[19, slots]^T @ table[19, 256] (counts built on host from int
    indices), followed by the GIN MLP on-device.
  - h1 is AllGather'd (fp16) across cores; layer 2 gathers h1[src] rows by
    indirect DMA and scatter-adds via one-hot matmuls in PSUM, followed by
    the layer-2 MLP.
  - Pooling back onto centers is a local one-hot matmul (indicator-sorted
    shards make center ranges disjoint per core).
  - BatchNorm stats (2x) via tiny AllReduce; projection + final norm +
    transpose on device. Host concatenates the 8 output slices.
All matmul operands fp16 (PE 1 cycle/row), accumulation fp32 in PSUM.
"""
import os
import sys

import numpy as np

sys.path.insert(0, "/opt/trn_rl_repo")

N = 20000
NSUB = 100000
ESUB = 300000
D = 256
EPS = 1e-5
NCORE = 8
CPC = N // NCORE            # 2500 centers per core
CPAD = 2560                 # padded to 20 blocks of 128
NBLK = CPAD // 128          # 20


# ----------------------------------------------------------------------------
# host preprocessing
# ----------------------------------------------------------------------------
def _preprocess(inputs):
    x = np.asarray(inputs["x"], np.int64)
    sni = np.asarray(inputs["subgraph_node_index"], np.int64)
    sei = np.asarray(inputs["subgraph_edge_index"], np.int64)
    sea = np.asarray(inputs["subgraph_edge_attr"], np.int64)
    sii = np.asarray(inputs["subgraph_indicator_index"], np.int64)

    pi = np.argsort(sii, kind="stable")
    inv = np.empty(NSUB, np.int64)
    inv[pi] = np.arange(NSUB)
    ind_s = sii[pi]
    node_s = sni[pi]

    src = inv[sei[0]]
    dst = inv[sei[1]]
    sl = np.arange(NSUB)
    src = np.concatenate([src, sl])
    dst = np.concatenate([dst, sl])
    ea0 = np.concatenate([sea[:, 0], np.full(NSUB, 4, np.int64)])
    ea1 = np.concatenate([sea[:, 1], np.zeros(NSUB, np.int64)])
    ecombo = np.where(ea0 == 4, 9, ea0 * 3 + ea1)
    ntype = x[node_s, 0] * 3 + x[node_s, 1]

    sub_lo = np.searchsorted(ind_s, np.arange(0, N + 1, CPC))
    core_of_pos = np.searchsorted(sub_lo, np.arange(NSUB), side="right") - 1
    blk_of_pos = (ind_s - core_of_pos * CPC) // 128
    cnt_cb = np.zeros((NCORE, NBLK), np.int64)
    np.add.at(cnt_cb, (core_of_pos, blk_of_pos), 1)
    S_max = int(np.ceil(cnt_cb.max() / 128))
    SLOTS = NBLK * S_max * 128

    # slot of each subgraph position: within (core, blk) in sorted order
    slot = np.zeros(NSUB, np.int64)
    # order within block: positions are already indicator-sorted per core
    key = core_of_pos * NBLK + blk_of_pos
    order = np.argsort(key, kind="stable")
    run_start = np.r_[0, np.flatnonzero(np.diff(key[order])) + 1]
    run_id = np.zeros(NSUB, np.int64)
    run_id[run_start] = 1
    run_id = np.cumsum(run_id) - 1
    within = np.arange(NSUB) - run_start[run_id]
    slot[order] = (blk_of_pos[order] * S_max * 128) + within
    gslot = core_of_pos * SLOTS + slot

    dst_core = core_of_pos[dst]
    ntile = SLOTS // 128
    dst_slot = slot[dst]
    dst_tile = dst_slot // 128
    e_cnt = np.zeros((NCORE, ntile), np.int64)
    np.add.at(e_cnt, (dst_core, dst_tile), 1)
    tiles_per_nt = np.ceil(e_cnt.max(axis=0) / 128).astype(np.int64)
    T_E = int(tiles_per_nt.sum())

    per_core = []
    for c in range(NCORE):
        em = dst_core == c
        cnt19 = np.zeros((19, SLOTS), np.float16)
        np.add.at(cnt19, (ntype[src[em]], dst_slot[em]), 1.0)
        np.add.at(cnt19, (9 + ecombo[em], dst_slot[em]), 1.0)

        gidx = np.zeros((T_E, 128), np.int32)
        oh_e = np.zeros((T_E, 128, 128), np.float16)
        tile_nt = np.zeros(T_E, np.int64)
        es_all, ed_all = src[em], dst_slot[em]
        o = np.argsort(ed_all, kind="stable")
        es_all, ed_all = es_all[o], ed_all[o]
        ed_tile = ed_all // 128
        bounds = np.searchsorted(ed_tile, np.arange(ntile + 1))
        t0 = 0
        for nt in range(ntile):
            a, b = bounds[nt], bounds[nt + 1]
            es, ed = es_all[a:b], ed_all[a:b]
            k = b - a
            for t in range(int(tiles_per_nt[nt])):
                lo2, hi2 = t * 128, min((t + 1) * 128, k)
                tile_nt[t0] = nt
                if hi2 > lo2:
                    m = hi2 - lo2
                    gidx[t0, :m] = gslot[es[lo2:hi2]]
                    oh_e[t0, np.arange(m), ed[lo2:hi2] - nt * 128] = 1.0
                t0 += 1
        assert t0 == T_E

        lo, hi = sub_lo[c], sub_lo[c + 1]
        oh_p = np.zeros((NBLK * S_max, 128, 128), np.float16)
        ind_l = np.full(SLOTS, -1, np.int64)
        ind_l[slot[lo:hi]] = ind_s[lo:hi] - c * CPC
        rows = np.arange(SLOTS)
        tiles = rows // 128
        cl = ind_l - (tiles // S_max) * 128
        valid = ind_l >= 0
        oh_p[tiles[valid], rows[valid] % 128, cl[valid]] = 1.0

        oh9 = np.zeros((9, CPAD), np.float16)
        cn = np.arange(c * CPC, (c + 1) * CPC)
        oh9[x[cn, 0] * 3 + x[cn, 1], np.arange(CPC)] = 1.0

        per_core.append(dict(cnt19=cnt19, gidx=gidx[:, :, None], oh_e=oh_e,
                             oh_p=oh_p, oh9=oh9))
    meta = dict(S_max=S_max, SLOTS=SLOTS, T_E=T_E, ntile=ntile,
                tiles_per_nt=[int(v) for v in tiles_per_nt])
    return per_core, meta


def _weight_maps(inputs):
    """Per-core replicated weight/constant tensors (host casts only)."""
    f16 = np.float16
    f32 = np.float32
    emb1 = np.asarray(inputs["emb1"], f32)
    emb2 = np.asarray(inputs["emb2"], f32)
    ee1 = np.asarray(inputs["edge_e1"], f32)
    ee2 = np.asarray(inputs["edge_e2"], f32)
    W1 = np.asarray(inputs["W1"], f32)
    b1 = np.asarray(inputs["b1"], f32)
    W2 = np.asarray(inputs["W2"], f32)
    b2 = np.asarray(inputs["b2"], f32)

    # selection matrices (constants): TAB1[t] = emb1[t//3]+emb2[t%3] (t<9),
    # TAB1[9+u] = ee1[0][bond(u)] + ee2[0][dir(u)], u<9 real, u=9 selfloop.
    sel1t = np.zeros((120, 19), f16)
    sel2t = np.zeros((3, 19), f16)
    selbt = np.zeros((6, 19), f16)
    seldt = np.zeros((3, 19), f16)
    for t in range(9):
        sel1t[t // 3, t] = 1
        sel2t[t % 3, t] = 1
    for u in range(9):
        selbt[u // 3, 9 + u] = 1
        seldt[u % 3, 9 + u] = 1
    selbt[4, 18] = 1
    seldt[0, 18] = 1
    selbt2 = np.zeros((6, 10), f16)
    seldt2 = np.zeros((3, 10), f16)
    for u in range(9):
        selbt2[u // 3, u] = 1
        seldt2[u % 3, u] = 1
    selbt2[4, 9] = 1
    seldt2[0, 9] = 1

    return dict(
        emb1f=emb1.astype(f16), emb2f=emb2.astype(f16),
        ee1a=ee1[0].astype(f16), ee1b=ee1[1].astype(f16),
        ee2a=ee2[0].astype(f16), ee2b=ee2[1].astype(f16),
        sel1t=sel1t, sel2t=sel2t, selbt=selbt, seldt=seldt,
        selbt2=selbt2, seldt2=seldt2,
        w1=W1.astype(f16), w2=W2.astype(f16),
        b1t=b1.reshape(2, 4, 128, 1).astype(f32),
        b2f=b2.reshape(2, 1, 256).astype(f16),
        wp=np.asarray(inputs["Wp"], f32).astype(f16),
        bpt=np.asarray(inputs["bp"], f32).reshape(2, 128, 1),
        bngt=np.asarray(inputs["bn_cat_g"], f32).reshape(4, 128, 1),
        bnbt=np.asarray(inputs["bn_cat_b"], f32).reshape(4, 128, 1),
        ngt=np.asarray(inputs["norm_g"], f32).reshape(2, 128, 1),
        nbt=np.asarray(inputs["norm_b"], f32).reshape(2, 128, 1),
    )


# ----------------------------------------------------------------------------
# bass kernel
# ----------------------------------------------------------------------------
def _build(meta):
    from concourse import bass, bacc, mybir, tile
    from concourse.masks import make_identity

    f16 = mybir.dt.float16
    f32 = mybir.dt.float32
    i32 = mybir.dt.int32
    AF = mybir.ActivationFunctionType
    OP = mybir.AluOpType

    SLOTS = meta["SLOTS"]
    T_E = meta["T_E"]
    NTILE = meta["ntile"]
    TPN = meta["tiles_per_nt"]
    S_max = meta["S_max"]

    nc = bacc.Bacc("TRN2", target_bir_lowering=False, debug=False,
                   num_devices=NCORE)

    def din(name, shape, dt):
        return nc.dram_tensor(name, shape, dt, kind="ExternalInput")

    cnt19 = din("cnt19", [19, SLOTS], f16)
    gidx = din("gidx", [T_E, 128, 1], i32)
    oh_e = din("oh_e", [T_E, 128, 128], f16)
    oh_p = din("oh_p", [NBLK * S_max, 128, 128], f16)
    oh9 = din("oh9", [9, CPAD], f16)
    emb1f = din("emb1f", [120, 256], f16)
    emb2f = din("emb2f", [3, 256], f16)
    ee1a = din("ee1a", [6, 256], f16)
    ee1b = din("ee1b", [6, 256], f16)
    ee2a = din("ee2a", [3, 256], f16)
    ee2b = din("ee2b", [3, 256], f16)
    sel1t = din("sel1t", [120, 19], f16)
    sel2t = din("sel2t", [3, 19], f16)
    selbt = din("selbt", [6, 19], f16)
    seldt = din("seldt", [3, 19], f16)
    selbt2 = din("selbt2", [6, 10], f16)
    seldt2 = din("seldt2", [3, 10], f16)
    w1 = din("w1", [2, 256, 512], f16)
    w2 = din("w2", [2, 512, 256], f16)
    b1t = din("b1t", [2, 4, 128, 1], f32)
    b2f = din("b2f", [2, 1, 256], f16)
    wp = din("wp", [512, 256], f16)
    bpt = din("bpt", [2, 128, 1], f32)
    bngt = din("bngt", [4, 128, 1], f32)
    bnbt = din("bnbt", [4, 128, 1], f32)
    ngt = din("ngt", [2, 128, 1], f32)
    nbt = din("nbt", [2, 128, 1], f32)
    out = nc.dram_tensor("out", [CPAD, 256], f32, kind="ExternalOutput")

    with tile.TileContext(nc) as tc:
        with (
            tc.tile_pool(name="const", bufs=1) as cpool,
            tc.tile_pool(name="wide", bufs=1) as wide,
            tc.tile_pool(name="work", bufs=3) as work,
            tc.tile_pool(name="mids", bufs=8) as midp,
            tc.tile_pool(name="aggp", bufs=4) as aggp,
            tc.tile_pool(name="statp", bufs=6) as statp,
            tc.tile_pool(name="msgs", bufs=8) as msgp,
            tc.tile_pool(name="ohs", bufs=8) as ohp,
            tc.tile_pool(name="h2s", bufs=NTILE) as h2p,
            tc.tile_pool(name="ps512", bufs=2, space="PSUM") as ps512,
            tc.tile_pool(name="ps256", bufs=2, space="PSUM") as ps256,
            tc.tile_pool(name="ps128", bufs=2, space="PSUM") as ps128,
            tc.tile_pool(name="dram", bufs=1, space="DRAM") as dram,
        ):
            # ---------------- constants / weights into SBUF ----------------
            _ldc = [0]

            def load(pool, src, shape, dt):
                _ldc[0] += 1
                nm = f"ld{_ldc[0]}"
                t = pool.tile(shape, dt, name=nm, tag=nm)
                nc.sync.dma_start(out=t[:], in_=src)
                return t

            sel1_sb = load(cpool, sel1t[:, :], [120, 19], f16)
            sel2_sb = load(cpool, sel2t[:, :], [3, 19], f16)
            selb_sb = load(cpool, selbt[:, :], [6, 19], f16)
            seld_sb = load(cpool, seldt[:, :], [3, 19], f16)
            selb2_sb = load(cpool, selbt2[:, :], [6, 10], f16)
            seld2_sb = load(cpool, seldt2[:, :], [3, 10], f16)
            emb1_sb = load(cpool, emb1f[:, :], [120, 256], f16)
            emb2_sb = load(cpool, emb2f[:, :], [3, 256], f16)
            ee1a_sb = load(cpool, ee1a[:, :], [6, 256], f16)
            ee1b_sb = load(cpool, ee1b[:, :], [6, 256], f16)
            ee2a_sb = load(cpool, ee2a[:, :], [3, 256], f16)
            ee2b_sb = load(cpool, ee2b[:, :], [3, 256], f16)
            w1_sb = [[load(cpool, w1[l, k * 128:(k + 1) * 128, :],
                           [128, 512], f16) for k in range(2)]
                     for l in range(2)]
            w2_sb = [[load(cpool, w2[l, k * 128:(k + 1) * 128, :],
                           [128, 256], f16) for k in range(4)]
                     for l in range(2)]
            wp_sb = [load(cpool, wp[k * 128:(k + 1) * 128, :],
                          [128, 256], f16) for k in range(4)]
            b1_sb = [[load(cpool, b1t[l, m], [128, 1], f32) for m in range(4)]
                     for l in range(2)]
            b2_sb = [load(cpool, b2f[l], [1, 256], f16) for l in range(2)]
            bp_sb = [load(cpool, bpt[c2], [128, 1], f32) for c2 in range(2)]
            bng_sb = [load(cpool, bngt[t], [128, 1], f32) for t in range(4)]
            bnb_sb = [load(cpool, bnbt[t], [128, 1], f32) for t in range(4)]
            ng_sb = [load(cpool, ngt[t], [128, 1], f32) for t in range(2)]
            nb_sb = [load(cpool, nbt[t], [128, 1], f32) for t in range(2)]
            oh9_sb = load(cpool, oh9[:, :], [9, CPAD], f16)

            ones_sb = cpool.tile([1, 128], f16)
            nc.vector.memset(ones_sb[:], 1.0)
            eps_sb = cpool.tile([128, 1], f32)
            nc.vector.memset(eps_sb[:], EPS)
            ident = cpool.tile([128, 128], f32)
            make_identity(nc, ident[:])

            # tables: TAB1 [19, 256] = sel1t^T@emb1 + sel2t^T@emb2 (+edge l0)
            tab_ps = ps256.tile([19, 256], f32, space="PSUM", tag="ps256")
            nc.tensor.matmul(tab_ps[:], lhsT=sel1_sb[:], rhs=emb1_sb[:],
                             start=True, stop=False)
            nc.tensor.matmul(tab_ps[:], lhsT=sel2_sb[:], rhs=emb2_sb[:],
                             start=False, stop=False)
            nc.tensor.matmul(tab_ps[:], lhsT=selb_sb[:], rhs=ee1a_sb[:],
                             start=False, stop=False)
            nc.tensor.matmul(tab_ps[:], lhsT=seld_sb[:], rhs=ee2a_sb[:],
                             start=False, stop=True)
            tab1_sb = cpool.tile([19, 256], f16)
            nc.vector.tensor_copy(out=tab1_sb[:], in_=tab_ps[:])

            tab2_ps = ps256.tile([10, 256], f32, space="PSUM", tag="ps256")
            nc.tensor.matmul(tab2_ps[:], lhsT=selb2_sb[:], rhs=ee1b_sb[:],
                             start=True, stop=False)
            nc.tensor.matmul(tab2_ps[:], lhsT=seld2_sb[:], rhs=ee2b_sb[:],
                             start=False, stop=True)
            ee2_sb = cpool.tile([10, 256], f16)
            nc.vector.tensor_copy(out=ee2_sb[:], in_=tab2_ps[:])

            # DRAM bounces
            h1loc = dram.tile([SLOTS, 256], f16)
            h1full = dram.tile([NCORE * SLOTS, 256], f16)
            st1loc = dram.tile([512, 2], f32)
            st1glob = dram.tile([512, 2], f32)
            st2loc = dram.tile([256, 2], f32)
            st2glob = dram.tile([256, 2], f32)

            # ---------------- shared MLP block (fm chunk of 512 rows) ------
            def mlp(l, agg_sb, row0, h_store):
                """agg_sb: 2 x [128, 512] f16 fm. Writes 4 row-tiles of 128
                rows (relu'd, f16, row-major [128, 256]) via h_store(r, tile).
                """
                mid_sb = []
                for m in range(4):
                    mp = ps512.tile([128, 512], f32, space="PSUM", tag="ps512")
                    for k in range(2):
                        nc.tensor.matmul(
                            mp[:],
                            lhsT=w1_sb[l][k][:, m * 128:(m + 1) * 128],
                            rhs=agg_sb[k][:], start=(k == 0), stop=(k == 1))
                    ms = midp.tile([128, 512], f16, tag="mid")
                    nc.scalar.activation(out=ms[:], in_=mp[:], func=AF.Relu,
                                         bias=b1_sb[l][m][:])
                    mid_sb.append(ms)
                for r in range(4):
                    hp = ps256.tile([128, 256], f32, space="PSUM", tag="ps256")
                    for k in range(4):
                        nc.tensor.matmul(
                            hp[:], lhsT=mid_sb[k][:, r * 128:(r + 1) * 128],
                            rhs=w2_sb[l][k][:],
                            start=(k == 0), stop=False)
                    nc.tensor.matmul(hp[:], lhsT=ones_sb[:], rhs=b2_sb[l][:],
                                     start=False, stop=True)
                    h_store(row0 + r, hp)

            # ---------------- phase B: layer 1 ----------------
            def store_h1(rt, hp):
                hs = work.tile([128, 256], f16, tag="h1out")
                nc.scalar.activation(out=hs[:], in_=hp[:], func=AF.Relu)
                nc.sync.dma_start(out=h1loc[rt * 128:(rt + 1) * 128, :],
                                  in_=hs[:])

            for ch in range(NTILE // 4):
                c0 = ch * 512
                cnt_sb = work.tile([19, 512], f16, tag="cnt")
                nc.sync.dma_start(out=cnt_sb[:],
                                  in_=cnt19[:, c0:c0 + 512])
                agg_sb = []
                for k in range(2):
                    ap_ = ps512.tile([128, 512], f32, space="PSUM",
                                     tag="ps512")
                    nc.tensor.matmul(ap_[:],
                                     lhsT=tab1_sb[:, k * 128:(k + 1) * 128],
                                     rhs=cnt_sb[:], start=True, stop=True)
                    asb = aggp.tile([128, 512], f16, tag="agg")
                    nc.vector.tensor_copy(out=asb[:], in_=ap_[:])
                    agg_sb.append(asb)
                mlp(0, agg_sb, ch * 4, store_h1)

            # ---------------- phase C: allgather h1 ----------------
            nc.gpsimd.collective_compute(
                "AllGather", OP.bypass,
                replica_groups=[list(range(NCORE))],
                ins=[h1loc[:].opt()], outs=[h1full[:].opt()])

            # ---------------- phase D: layer 2 ----------------
            h2_tiles = []

            def store_h2(rt, hp):
                hs = h2p.tile([128, 256], f16, tag="h2")
                nc.scalar.activation(out=hs[:], in_=hp[:], func=AF.Relu)
                h2_tiles.append(hs)

            t0 = 0
            for ch in range(NTILE // 4):
                agg_sb = [aggp.tile([128, 512], f16, tag="agg2",
                                    name=f"agg2sb{_k}") for _k in range(2)]
                for j in range(4):
                    nt = ch * 4 + j
                    cntE_sb = work.tile([10, 128], f16, tag="cntE")
                    nc.sync.dma_start(
                        out=cntE_sb[:],
                        in_=cnt19[9:19, nt * 128:(nt + 1) * 128])
                    gps = [ps128.tile([128, 128], f32, space="PSUM",
                                      tag=f"g{k}", name=f"gps{k}")
                           for k in range(2)]
                    for k in range(2):
                        nc.tensor.matmul(
                            gps[k][:], lhsT=ee2_sb[:, k * 128:(k + 1) * 128],
                            rhs=cntE_sb[:], start=True,
                            stop=(TPN[nt] == 0))
                    for t in range(t0, t0 + TPN[nt]):
                        idx_sb = work.tile([128, 1], i32, tag="idx")
                        nc.sync.dma_start(out=idx_sb[:], in_=gidx[t])
                        msg_sb = msgp.tile([128, 256], f16, tag="msg")
                        nc.gpsimd.indirect_dma_start(
                            out=msg_sb[:], out_offset=None,
                            in_=h1full[:],
                            in_offset=bass.IndirectOffsetOnAxis(
                                ap=idx_sb[:, :1], axis=0))
                        oh_sb = ohp.tile([128, 128], f16, tag="oh")
                        nc.sync.dma_start(out=oh_sb[:], in_=oh_e[t])
                        last = t == t0 + TPN[nt] - 1
                        for k in range(2):
                            nc.tensor.matmul(
                                gps[k][:],
                                lhsT=msg_sb[:, k * 128:(k + 1) * 128],
                                rhs=oh_sb[:], start=False, stop=last)
                    t0 += TPN[nt]
                    for k in range(2):
                        nc.vector.tensor_copy(
                            out=agg_sb[k][:, j * 128:(j + 1) * 128],
                            in_=gps[k][:])
                mlp(1, agg_sb, ch * 4, store_h2)
            assert t0 == T_E

            # ---------------- phase E: pooling + origin -> cat fm ----------
            cat_sb = [wide.tile([128, CPAD], f32, tag=f"cat{t}",
                                name=f"cat{t}") for t in range(4)]
            for k in range(2):
                for w in range(CPAD // 512):
                    op_ = ps512.tile([128, 512], f32, space="PSUM",
                                     tag="ps512")
                    nc.tensor.matmul(
                        op_[:], lhsT=tab1_sb[0:9, k * 128:(k + 1) * 128],
                        rhs=oh9_sb[:, w * 512:(w + 1) * 512],
                        start=True, stop=True)
                    nc.vector.tensor_copy(
                        out=cat_sb[k][:, w * 512:(w + 1) * 512], in_=op_[:])
            for b in range(NBLK):
                pps = [ps128.tile([128, 128], f32, space="PSUM",
                                  tag=f"g{k}", name=f"pps{k}")
                       for k in range(2)]
                for s in range(S_max):
                    ohp_sb = ohp.tile([128, 128], f16, tag="ohp")
                    nc.sync.dma_start(out=ohp_sb[:], in_=oh_p[b * S_max + s])
                    for k in range(2):
                        nc.tensor.matmul(
                            pps[k][:],
                            lhsT=h2_tiles[b * S_max + s][:,
                                                         k * 128:(k + 1) * 128],
                            rhs=ohp_sb[:], start=(s == 0),
                            stop=(s == S_max - 1))
                for k in range(2):
                    nc.vector.tensor_copy(
                        out=cat_sb[2 + k][:, b * 128:(b + 1) * 128],
                        in_=pps[k][:])

            # ---------------- phase F: BN1 -> proj -> BN2 -> out -----------
            def stats(tiles, n_real, loc, glob, nt_):
                s_sbs = []
                for t in range(nt_):
                    s_sb = statp.tile([128, 2], f32, tag="stat")
                    nc.vector.tensor_reduce(
                        out=s_sb[:, 0:1], in_=tiles[t][:, 0:n_real],
                        axis=mybir.AxisListType.X, op=OP.add)
                    sq = wide.tile([128, CPC], f32, tag="sqtmp")
                    nc.vector.tensor_tensor(
                        out=sq[:], in0=tiles[t][:, 0:n_real],
                        in1=tiles[t][:, 0:n_real], op=OP.mult)
                    nc.vector.tensor_reduce(
                        out=s_sb[:, 1:2], in_=sq[:],
                        axis=mybir.AxisListType.X, op=OP.add)
                    nc.sync.dma_start(out=loc[t * 128:(t + 1) * 128, :],
                                      in_=s_sb[:])
                    s_sbs.append(s_sb)
                nc.gpsimd.collective_compute(
                    "AllReduce", OP.add,
                    replica_groups=[list(range(NCORE))],
                    ins=[loc[:].opt()], outs=[glob[:].opt()])
                outs = []
                for t in range(nt_):
                    g_sb = statp.tile([128, 2], f32, tag="gstat")
                    nc.sync.dma_start(out=g_sb[:],
                                      in_=glob[t * 128:(t + 1) * 128, :])
                    outs.append(g_sb)
                return outs

            def scale_bias(g_sb, gam, bet):
                # mu = s0/N; var = s1/N - mu^2; rstd = 1/sqrt(var+eps)
                mu = work.tile([128, 1], f32, tag="mu")
                nc.vector.tensor_scalar_mul(mu[:], g_sb[:, 0:1], 1.0 / N)
                var = work.tile([128, 1], f32, tag="var")
                nc.vector.tensor_scalar_mul(var[:], g_sb[:, 1:2], 1.0 / N)
                musq = work.tile([128, 1], f32, tag="musq")
                nc.vector.tensor_tensor(out=musq[:], in0=mu[:], in1=mu[:],
                                        op=OP.mult)
                nc.vector.tensor_tensor(out=var[:], in0=var[:], in1=musq[:],
                                        op=OP.subtract)
                sd = work.tile([128, 1], f32, tag="sd")
                nc.scalar.activation(out=sd[:], in_=var[:], func=AF.Sqrt,
                                     bias=eps_sb[:, 0:1])
                rstd = work.tile([128, 1], f32, tag="rstd")
                nc.vector.reciprocal(rstd[:], sd[:])
                sc = work.tile([128, 1], f32, tag="sc")
                nc.vector.tensor_tensor(out=sc[:], in0=rstd[:], in1=gam[:],
                                        op=OP.mult)
                bi = work.tile([128, 1], f32, tag="bi")
                nc.vector.tensor_tensor(out=bi[:], in0=mu[:], in1=sc[:],
                                        op=OP.mult)
                nc.vector.tensor_tensor(out=bi[:], in0=bet[:], in1=bi[:],
                                        op=OP.subtract)
                return sc, bi

            g1 = stats(cat_sb, CPC, st1loc, st1glob, 4)
            bn_sb = []
            for t in range(4):
                sc, bi = scale_bias(g1[t], bng_sb[t], bnb_sb[t])
                bt = wide.tile([128, CPAD], f16, tag=f"bn{t}")
                nc.vector.tensor_scalar(
                    out=bt[:], in0=cat_sb[t][:], scalar1=sc[:, 0:1],
                    scalar2=bi[:, 0:1], op0=OP.mult, op1=OP.add)
                bn_sb.append(bt)

            out2_sb = [wide.tile([128, CPAD], f32, tag=f"o2_{c2}",
                                 name=f"o2sb{c2}") for c2 in range(2)]
            for w in range(CPAD // 512):
                for c2 in range(2):
                    pp = ps512.tile([128, 512], f32, space="PSUM",
                                     tag="ps512")
                    for k in range(4):
                        nc.tensor.matmul(
                            pp[:],
                            lhsT=wp_sb[k][:, c2 * 128:(c2 + 1) * 128],
                            rhs=bn_sb[k][:, w * 512:(w + 1) * 512],
                            start=(k == 0), stop=(k == 3))
                    nc.vector.tensor_scalar(
                        out=out2_sb[c2][:, w * 512:(w + 1) * 512], in0=pp[:],
                        scalar1=bp_sb[c2][:, 0:1], scalar2=None, op0=OP.add)

            g2 = stats(out2_sb, CPC, st2loc, st2glob, 2)
            for c2 in range(2):
                sc, bi = scale_bias(g2[c2], ng_sb[c2], nb_sb[c2])
                nc.vector.tensor_scalar(
                    out=out2_sb[c2][:], in0=out2_sb[c2][:], scalar1=sc[:, 0:1],
                    scalar2=bi[:, 0:1], op0=OP.mult, op1=OP.add)

            for w in range(NBLK):
                os_ = work.tile([128, 256], f32, tag="outrm")
                for c2 in range(2):
                    tp = ps128.tile([128, 128], f32, space="PSUM", tag="g0")
                    nc.tensor.transpose(
                        out=tp[:], in_=out2_sb[c2][:, w * 128:(w + 1) * 128],
                        identity=ident[:])
                    nc.vector.tensor_copy(
                        out=os_[:, c2 * 128:(c2 + 1) * 128], in_=tp[:])
                nc.sync.dma_start(out=out[w * 128:(w + 1) * 128, :],
                                  in_=os_[:])

    nc.compile()
    return nc


_CACHE = {}


def kernel(**inputs):
    from concourse.bass_utils import run_bass_kernel_spmd

    per_core, meta = _preprocess(inputs)
    wm = _weight_maps(inputs)

    key = (meta["SLOTS"], meta["T_E"], tuple(meta["tiles_per_nt"]))
    if key not in _CACHE:
        _CACHE[key] = _build(meta)
    nc = _CACHE[key]

    in_maps = []
    for c in range(NCORE):
        m = dict(per_core[c])
        m.update(wm)
        in_maps.append(m)

    trace = bool(int(os.environ.get("KERNEL_TRACE", "0")))
    res = run_bass_kernel_spmd(nc, in_maps, list(range(NCORE)), trace=trace)
    kernel.last_results = res

    outs = [res.results[c]["out"][:CPC] for c in range(NCORE)]
    return np.concatenate(outs, 0).astype(np.float32)



# revision 31
# speedup vs baseline: 1.7862x; 1.7862x over previous
"""Trainium2 Bass kernel for the GNN k-hop subgraph encoder (GIN, L=2, D=256).

Strategy (8 cores, graph-parallel, v2):
  - Host: sort subgraph nodes by indicator (center id); shard by center
    (2500 centers/core); bin-pack centers into 20 blocks of <=128 centers
    and <=S*128 slots (S=5 typically) -> ROWS = 20*S*128 tight node rows.
  - Layer 1 needs NO gather: layer-1 aggregation is counts[19, ROWS]^T @
    table[19, 256] (counts built on host), then the GIN MLP on device.
  - h1 is stored fp8 (e4m3) and AllGather'd in NCHUNK=4 chunks overlapped
    with layer-1 compute; layer 2 gathers h1[src] rows with dma_gather
    (thousands of rows per instruction, 4 address-region classes to fit
    int16 indices), scatter-adds via one-hot matmuls in PSUM where the
    one-hots are generated on-device (iota/is_equal on the vector engine).
    Self-loops skip the gather entirely: identity matmul from the fp16 h1
    kept in SBUF.
  - Pooling onto centers is an incremental one-hot matmul per block;
    BatchNorm stats via one AllReduce (origin-half stats computed early);
    projection + final norm on device; output stays feature-major and the
    host transposes/un-permutes.
All matmul operands fp16/fp8, accumulation fp32 in PSUM.
"""
import os
import sys

import numpy as np

sys.path.insert(0, "/opt/trn_rl_repo")

N = 20000
NSUB = 100000
ESUB = 300000
D = 256
EPS = 1e-5
NCORE = 8
CPC = N // NCORE            # 2500 centers per core
NBLK = 20                   # center blocks of 128
CPAD = NBLK * 128           # 2560
NCHUNK = 4                  # AllGather chunks == gather region classes
GB_NT = 4                   # node tiles per gather block
H1_FP8 = False
DBG_TAPS = False


# ----------------------------------------------------------------------------
# host preprocessing
# ----------------------------------------------------------------------------
def _pack_centers(counts, cap_slots):
    """Greedy least-loaded packing of centers into NBLK blocks.
    counts: [CPC] slots per center. Returns blocks: list of NBLK lists of
    center ids, or None if infeasible under (cap_slots, 128 centers)."""
    order = np.argsort(-counts, kind="stable")
    loads = np.zeros(NBLK, np.int64)
    ncent = np.zeros(NBLK, np.int64)
    blocks = [[] for _ in range(NBLK)]
    for ctr in order:
        k = counts[ctr]
        # least-loaded block with room
        best, bestload = -1, None
        for b in range(NBLK):
            if ncent[b] < 128 and loads[b] + k <= cap_slots:
                if bestload is None or loads[b] < bestload:
                    best, bestload = b, loads[b]
        if best < 0:
            return None
        blocks[best].append(int(ctr))
        loads[best] += k
        ncent[best] += 1
    return blocks


def _preprocess(inputs):
    x = np.asarray(inputs["x"], np.int64)
    sni = np.asarray(inputs["subgraph_node_index"], np.int64)
    sei = np.asarray(inputs["subgraph_edge_index"], np.int64)
    sea = np.asarray(inputs["subgraph_edge_attr"], np.int64)
    sii = np.asarray(inputs["subgraph_indicator_index"], np.int64)

    pi = np.argsort(sii, kind="stable")
    ind_s = sii[pi]
    node_s = sni[pi]
    sub_lo = np.searchsorted(ind_s, np.arange(0, N + 1, CPC))

    # --- per-center slot counts, bin-pack into blocks -----------------------
    ctr_cnt = np.zeros(N, np.int64)
    np.add.at(ctr_cnt, ind_s, 1)
    ctr_cnt = ctr_cnt.reshape(NCORE, CPC)

    S = 5
    packs = []
    for c in range(NCORE):
        blocks = _pack_centers(ctr_cnt[c], S * 128)
        if blocks is None:
            S = 6
            packs = []
            for c2 in range(NCORE):
                blocks = _pack_centers(ctr_cnt[c2], S * 128)
                assert blocks is not None, "center packing failed at S=6"
                packs.append(blocks)
            break
        packs.append(blocks)
    ROWS = NBLK * S * 128
    NTILE = ROWS // 128
    CH = ROWS // NCHUNK
    REG = NCORE * CH
    assert REG <= 32767 and ROWS % 512 == 0

    # --- slot assignment ----------------------------------------------------
    # pos -> (core, slot); slot layout: block b owns [b*S*128, (b+1)*S*128)
    slot_of_pos = np.zeros(NSUB, np.int64)
    indloc = np.full((NCORE, ROWS), -1, np.int64)   # slot -> center-local col
    colmap = np.full((NCORE, CPAD), -1, np.int64)   # cat col -> global center
    ctr_start = np.zeros(N + 1, np.int64)           # run start of each center
    np.cumsum(ctr_cnt.reshape(-1), out=ctr_start[1:])
    for c in range(NCORE):
        for b in range(NBLK):
            off = b * S * 128
            for j, ctr in enumerate(packs[c][b]):
                g = c * CPC + ctr
                lo, hi = ctr_start[g], ctr_start[g + 1]
                k = hi - lo
                slot_of_pos[pi[lo:hi]] = off + np.arange(k)
                indloc[c, off:off + k] = j
                colmap[c, b * 128 + j] = g
                off += k
    cps = np.searchsorted(sub_lo, np.arange(NSUB), side="right") - 1
    core_of_pos = np.empty(NSUB, np.int64)
    core_of_pos[pi] = cps

    # --- L1 count matrix (includes self-loops) ------------------------------
    ntype = x[node_s, 0] * 3 + x[node_s, 1]
    # re-map to per-position arrays in original position index space
    ntype_pos = np.zeros(NSUB, np.int64)
    ntype_pos[pi] = ntype
    src = sei[0]
    dst = sei[1]
    ecombo = sea[:, 0] * 3 + sea[:, 1]

    cnt19 = np.zeros((NCORE, 19, ROWS), np.float32)
    dcore = core_of_pos[dst]
    dslot = slot_of_pos[dst]
    np.add.at(cnt19, (dcore, ntype_pos[src], dslot), 1.0)
    np.add.at(cnt19, (dcore, 9 + ecombo, dslot), 1.0)
    # self loops: type of self + combo 9
    score = core_of_pos[np.arange(NSUB)]
    sslot = slot_of_pos[np.arange(NSUB)]
    np.add.at(cnt19, (score, ntype_pos[np.arange(NSUB)], sslot), 1.0)
    np.add.at(cnt19, (score, np.full(NSUB, 18), sslot), 1.0)

    # --- gather address of each position ------------------------------------
    # h1full layout: [chunk][core][CH rows]
    gaddr_chunk = slot_of_pos // CH
    gaddr_idx = core_of_pos * CH + slot_of_pos % CH   # index within region

    # --- edge tiling: per (nt, cls) lists, shared Tmax ----------------------
    # self-loops ride the gather stream too (h1[self] into agg2)
    sl = np.arange(NSUB)
    src2 = np.concatenate([src, sl])
    dst2 = np.concatenate([dst, sl])
    dcore2 = core_of_pos[dst2]
    dslot2 = slot_of_pos[dst2]
    scls = gaddr_chunk[src2]
    sidx = gaddr_idx[src2]
    dnt = dslot2 // 128
    dloc = dslot2 % 128
    ecnt = np.zeros((NCORE, NTILE, NCHUNK), np.int64)
    np.add.at(ecnt, (dcore2, dnt, scls), 1)
    Tmax = np.ceil(ecnt.max(axis=0) / 128).astype(np.int64)  # [NTILE, NCHUNK]

    NGB = NTILE // GB_NT
    assert NTILE % GB_NT == 0
    T_E = int(Tmax.sum())
    # tile/program ordering: gb -> cls -> nt in gb -> tile
    tile_off = np.zeros((NTILE, NCHUNK), np.int64)  # global tile id of first
    iw_off = np.zeros((NGB, NCHUNK), np.int64)      # idx col offset per instr
    iw_len = np.zeros((NGB, NCHUNK), np.int64)      # num_idxs per instr
    t0 = 0
    col0 = 0
    for gb in range(NGB):
        for cls in range(NCHUNK):
            n_idx = 0
            for nt in range(gb * GB_NT, (gb + 1) * GB_NT):
                tile_off[nt, cls] = t0
                t0 += Tmax[nt, cls]
                n_idx += int(Tmax[nt, cls]) * 128
            iw_off[gb, cls] = col0
            iw_len[gb, cls] = n_idx
            col0 += n_idx // 16
    assert t0 == T_E
    IDXCOLS = col0

    per_core = []
    for c in range(NCORE):
        em = dcore2 == c
        es_cls, es_idx = scls[em], sidx[em]
        es_nt, es_loc = dnt[em], dloc[em]
        order = np.argsort(es_nt * NCHUNK + es_cls, kind="stable")
        es_cls, es_idx = es_cls[order], es_idx[order]
        es_nt, es_loc = es_nt[order], es_loc[order]
        bounds = np.searchsorted(
            es_nt * NCHUNK + es_cls, np.arange(NTILE * NCHUNK + 1))

        gidx16 = np.zeros((128, IDXCOLS), np.int16)
        dstl = np.full((T_E, 128), -1.0, np.float32)
        for nt in range(NTILE):
            for cls in range(NCHUNK):
                a, b = bounds[nt * NCHUNK + cls], bounds[nt * NCHUNK + cls + 1]
                k = b - a
                cap = int(Tmax[nt, cls]) * 128
                assert k <= cap, (c, nt, cls, k, cap)
                vals = np.zeros(cap, np.int16)
                vals[:k] = es_idx[a:b]
                tg = tile_off[nt, cls]
                dstl[tg:tg + cap // 128] = -1.0
                dl = np.full(cap, -1.0, np.float32)
                dl[:k] = es_loc[a:b]
                dstl[tg:tg + cap // 128] = dl.reshape(-1, 128)
                # idx columns: within instruction (gb, cls)
                gb = nt // GB_NT
                # offset of this nt's idxs inside the instruction
                pre = int((Tmax[nt // GB_NT * GB_NT:nt, cls]).sum()) * 128
                cstart = int(iw_off[gb, cls]) + pre // 16
                arr16 = vals.reshape(-1, 16).T  # [16, cap/16]
                gidx16[:, cstart:cstart + cap // 16] = np.tile(arr16, (8, 1))

        oh9 = np.zeros((9, CPAD), np.float32)
        cols = np.flatnonzero(colmap[c] >= 0)
        gctr = colmap[c][cols]
        oh9[x[gctr, 0] * 3 + x[gctr, 1], cols] = 1.0

        il = np.zeros((128, NTILE), np.float32)
        il[:, :] = indloc[c].reshape(NTILE, 128).T
        per_core.append(dict(
            cnt19=cnt19[c].astype(np.float16),
            cnt10=cnt19[c, 9:19].astype(np.float16),
            gidx16=gidx16,
            dstl=np.ascontiguousarray(dstl.T.astype(np.float32)),
            indloc=il.astype(np.float32),
            oh9=oh9.astype(np.float16),
        ))

    GMAX = 0
    for gb in range(NGB):
        for cls in range(NCHUNK):
            GMAX = max(GMAX, int(iw_len[gb, cls]) // 128)
    meta = dict(S=S, ROWS=ROWS, NTILE=NTILE, CH=CH, REG=REG, NGB=NGB,
                T_E=T_E, IDXCOLS=IDXCOLS, GMAX=GMAX,
                Tmax=[[int(v) for v in row] for row in Tmax],
                tile_off=[[int(v) for v in row] for row in tile_off],
                iw_off=[[int(v) for v in row] for row in iw_off],
                iw_len=[[int(v) for v in row] for row in iw_len])
    return per_core, meta, colmap


def _weight_maps(inputs):
    f16 = np.float16
    f32 = np.float32
    emb1 = np.asarray(inputs["emb1"], f32)
    emb2 = np.asarray(inputs["emb2"], f32)
    ee1 = np.asarray(inputs["edge_e1"], f32)
    ee2 = np.asarray(inputs["edge_e2"], f32)

    # TAB1[t] = emb1[t//3]+emb2[t%3] (t<9); TAB1[9+u] = ee1[0][b]+ee2[0][d]
    # (u<9: b=u//3, d=u%3; u=9: self loop b=4, d=0)
    tab1 = np.zeros((19, D), f32)
    for t in range(9):
        tab1[t] = emb1[t // 3] + emb2[t % 3]
    for u in range(9):
        tab1[9 + u] = ee1[0][u // 3] + ee2[0][u % 3]
    tab1[18] = ee1[0][4] + ee2[0][0]
    ee2t = np.zeros((10, D), f32)
    for u in range(9):
        ee2t[u] = ee1[1][u // 3] + ee2[1][u % 3]
    ee2t[9] = ee1[1][4] + ee2[1][0]

    colidx = np.tile(np.arange(128, dtype=f32), (128, 1))

    return dict(
        tab1=tab1.astype(f16), ee2t=ee2t.astype(f16),
        colidx=colidx.astype(f16),
        w1=np.asarray(inputs["W1"], f32).astype(f16),
        w2=np.asarray(inputs["W2"], f32).astype(f16),
        b1t=np.asarray(inputs["b1"], f32).reshape(2, 4, 128, 1),
        b2f=np.asarray(inputs["b2"], f32).reshape(2, 1, 256).astype(f16),
        wp=np.asarray(inputs["Wp"], f32),
        bpt=np.asarray(inputs["bp"], f32).reshape(2, 128, 1),
        bngt=np.asarray(inputs["bn_cat_g"], f32).reshape(4, 128, 1),
        bnbt=np.asarray(inputs["bn_cat_b"], f32).reshape(4, 128, 1),
        ngt=np.asarray(inputs["norm_g"], f32).reshape(2, 128, 1),
        nbt=np.asarray(inputs["norm_b"], f32).reshape(2, 128, 1),
    )


# ----------------------------------------------------------------------------
# bass kernel
# ----------------------------------------------------------------------------
def _build(meta):
    from concourse import bass, bacc, mybir, tile
    from concourse.masks import make_identity

    f16 = mybir.dt.float16
    f32 = mybir.dt.float32
    f8 = mybir.dt.float8e4
    i16 = mybir.dt.int16
    AF = mybir.ActivationFunctionType
    OP = mybir.AluOpType

    H1DT = f8 if H1_FP8 else f16
    H1SZ = 1 if H1_FP8 else 2

    S = meta["S"]
    ROWS = meta["ROWS"]
    NTILE = meta["NTILE"]
    CH = meta["CH"]
    REG = meta["REG"]
    NGB = meta["NGB"]
    T_E = meta["T_E"]
    IDXCOLS = meta["IDXCOLS"]
    Tmax = meta["Tmax"]
    tile_off = meta["tile_off"]
    iw_off = meta["iw_off"]
    iw_len = meta["iw_len"]
    NPAD = CPAD - CPC  # padded (fake) center columns per core

    nc = bacc.Bacc("TRN2", target_bir_lowering=False, debug=False,
                   num_devices=NCORE)

    def din(name, shape, dt):
        return nc.dram_tensor(name, shape, dt, kind="ExternalInput")

    cnt19 = din("cnt19", [19, ROWS], f16)
    cnt10 = din("cnt10", [10, ROWS], f16)
    gidx16 = din("gidx16", [128, IDXCOLS], i16)
    dstl = din("dstl", [128, T_E], f32)
    indloc = din("indloc", [128, NTILE], f32)
    oh9 = din("oh9", [9, CPAD], f16)
    tab1 = din("tab1", [19, 256], f16)
    ee2t = din("ee2t", [10, 256], f16)
    colidx = din("colidx", [128, 128], f16)
    w1 = din("w1", [2, 256, 512], f16)
    w2 = din("w2", [2, 512, 256], f16)
    b1t = din("b1t", [2, 4, 128, 1], f32)
    b2f = din("b2f", [2, 1, 256], f16)
    wp = din("wp", [512, 256], f32)
    bpt = din("bpt", [2, 128, 1], f32)
    bngt = din("bngt", [4, 128, 1], f32)
    bnbt = din("bnbt", [4, 128, 1], f32)
    ngt = din("ngt", [2, 128, 1], f32)
    nbt = din("nbt", [2, 128, 1], f32)
    out = nc.dram_tensor("out", [2, 128, CPAD], f32, kind="ExternalOutput")
    if DBG_TAPS:
        dbg_agg2 = nc.dram_tensor("dbg_agg2", [2, 128, ROWS], f16,
                                  kind="ExternalOutput")
        dbg_cat = nc.dram_tensor("dbg_cat", [4, 128, CPAD], f32,
                                 kind="ExternalOutput")
        dbg_h1 = nc.dram_tensor("dbg_h1", [ROWS, 256], f16,
                                kind="ExternalOutput")

    with tile.TileContext(nc) as tc:
        with (
            tc.tile_pool(name="const", bufs=1) as cpool,
            tc.tile_pool(name="wide", bufs=1) as wide,
            tc.tile_pool(name="work", bufs=4) as work,
            tc.tile_pool(name="aggp", bufs=3) as aggp,
            tc.tile_pool(name="mids", bufs=6) as midp,
            tc.tile_pool(name="statp", bufs=8) as statp,
            tc.tile_pool(name="ohs", bufs=14) as ohp,
            tc.tile_pool(name="h1w", bufs=4) as h1wp,
            tc.tile_pool(name="h2s", bufs=4 + 2 * S) as h2p,
            tc.tile_pool(name="gout", bufs=2) as goutp,
            tc.tile_pool(name="ps512", bufs=2, space="PSUM") as ps512,
            tc.tile_pool(name="ps256", bufs=2, space="PSUM") as ps256,
            tc.tile_pool(name="psg", bufs=2, space="PSUM") as psg,
            tc.tile_pool(name="psp", bufs=2, space="PSUM") as psp,
            tc.tile_pool(name="dram", bufs=1, space="DRAM") as dram,
        ):
            # ---------------- constants / weights into SBUF ----------------
            _ldc = [0]

            def load(pool, eng, src, shape, dt):
                _ldc[0] += 1
                nm = f"ld{_ldc[0]}"
                t = pool.tile(shape, dt, name=nm, tag=nm)
                eng.dma_start(out=t[:], in_=src)
                return t

            cnt10_sb = load(cpool, nc.sync, cnt10[:, :], [10, ROWS], f16)
            gidx_sb = load(cpool, nc.scalar, gidx16[:, :], [128, IDXCOLS], i16)
            dstl_sb = load(cpool, nc.scalar, dstl[:, :], [128, T_E], f32)
            indloc_sb = load(cpool, nc.scalar, indloc[:, :], [128, NTILE], f32)
            oh9_sb = load(cpool, nc.scalar, oh9[:, :], [9, CPAD], f16)
            tab1_sb = load(cpool, nc.scalar, tab1[:, :], [19, 256], f16)
            ee2_sb = load(cpool, nc.scalar, ee2t[:, :], [10, 256], f16)
            colidx_sb = load(cpool, nc.scalar, colidx[:, :], [128, 128], f16)
            w1_sb = [[load(cpool, nc.scalar, w1[l, k * 128:(k + 1) * 128, :],
                           [128, 512], f16) for k in range(2)]
                     for l in range(2)]
            w2_sb = [[load(cpool, nc.scalar, w2[l, k * 128:(k + 1) * 128, :],
                           [128, 256], f16) for k in range(4)]
                     for l in range(2)]
            wp_sb = [load(cpool, nc.scalar, wp[k * 128:(k + 1) * 128, :],
                          [128, 256], f32) for k in range(4)]
            b1_sb = [[load(cpool, nc.scalar, b1t[l, m], [128, 1], f32)
                      for m in range(4)] for l in range(2)]
            b2_sb = [load(cpool, nc.scalar, b2f[l], [1, 256], f16)
                     for l in range(2)]
            bp_sb = [load(cpool, nc.scalar, bpt[c2], [128, 1], f32)
                     for c2 in range(2)]
            bng_sb = [load(cpool, nc.scalar, bngt[t], [128, 1], f32)
                      for t in range(4)]
            bnb_sb = [load(cpool, nc.scalar, bnbt[t], [128, 1], f32)
                      for t in range(4)]
            ng_sb = [load(cpool, nc.scalar, ngt[t], [128, 1], f32)
                     for t in range(2)]
            nb_sb = [load(cpool, nc.scalar, nbt[t], [128, 1], f32)
                     for t in range(2)]

            ones_sb = cpool.tile([1, 128], f16)
            nc.vector.memset(ones_sb[:], 1.0)
            eps_sb = cpool.tile([128, 1], f32)
            nc.vector.memset(eps_sb[:], EPS)

            # DRAM bounces
            h1loc = dram.tile([ROWS, 256], H1DT)
            h1full = [dram.tile([NCORE * CH, 256], H1DT,
                                addr_space="Shared", name=f"h1full{cgi}",
                                tag=f"h1full{cgi}")
                      for cgi in range(NCHUNK)]
            st1loc = dram.tile([512, 2], f32)
            st1glob = dram.tile([512, 2], f32)
            st2loc = dram.tile([256, 2], f32)
            st2glob = dram.tile([256, 2], f32)

            # ---------------- phase A: origin half of cat + stats ----------
            cat_sb = [wide.tile([128, CPAD], f32, tag=f"cat{t}",
                                name=f"cat{t}") for t in range(4)]
            for k in range(2):
                for w in range(CPAD // 512):
                    op_ = ps512.tile([128, 512], f32, space="PSUM",
                                     tag="ps512")
                    nc.tensor.matmul(
                        op_[:], lhsT=tab1_sb[0:9, k * 128:(k + 1) * 128],
                        rhs=oh9_sb[:, w * 512:(w + 1) * 512],
                        start=True, stop=True)
                    nc.vector.tensor_copy(
                        out=cat_sb[k][:, w * 512:(w + 1) * 512], in_=op_[:])

            sqtmp = wide.tile([128, 512], f32, tag="sqtmp", name="sqtmp")

            def tile_stats(src_sb, loc, row0, t):
                s_sb = statp.tile([128, 2], f32, tag="stat")
                nc.vector.tensor_reduce(
                    out=s_sb[:, 0:1], in_=src_sb[:],
                    axis=mybir.AxisListType.X, op=OP.add)
                s1t = statp.tile([128, 1], f32, tag="s1t")
                for w in range(CPAD // 512):
                    nc.vector.tensor_tensor(
                        out=sqtmp[:], in0=src_sb[:, w * 512:(w + 1) * 512],
                        in1=src_sb[:, w * 512:(w + 1) * 512], op=OP.mult)
                    if w == 0:
                        nc.vector.tensor_reduce(
                            out=s_sb[:, 1:2], in_=sqtmp[:],
                            axis=mybir.AxisListType.X, op=OP.add)
                    else:
                        nc.vector.tensor_reduce(
                            out=s1t[:], in_=sqtmp[:],
                            axis=mybir.AxisListType.X, op=OP.add)
                        nc.vector.tensor_tensor(
                            out=s_sb[:, 1:2], in0=s_sb[:, 1:2], in1=s1t[:],
                            op=OP.add)
                nc.sync.dma_start(
                    out=loc[(row0 + t) * 128:(row0 + t + 1) * 128, :],
                    in_=s_sb[:])

            for t in range(2):
                tile_stats(cat_sb[t], st1loc, 0, t)

            # ---------------- shared MLP block (512 rows) -------------------
            def mlp(l, agg_sb, row0, h_store):
                mid_sb = []
                for m in range(4):
                    mp = ps512.tile([128, 512], f32, space="PSUM", tag="ps512")
                    for k in range(2):
                        nc.tensor.matmul(
                            mp[:],
                            lhsT=w1_sb[l][k][:, m * 128:(m + 1) * 128],
                            rhs=agg_sb[k][:], start=(k == 0), stop=(k == 1))
                    ms = midp.tile([128, 512], f16, tag="mid")
                    nc.scalar.activation(out=ms[:], in_=mp[:], func=AF.Relu,
                                         bias=b1_sb[l][m][:])
                    mid_sb.append(ms)
                for r in range(4):
                    hp = ps256.tile([128, 256], f32, space="PSUM", tag="ps256")
                    for k in range(4):
                        nc.tensor.matmul(
                            hp[:], lhsT=mid_sb[k][:, r * 128:(r + 1) * 128],
                            rhs=w2_sb[l][k][:], start=(k == 0), stop=False)
                    nc.tensor.matmul(hp[:], lhsT=ones_sb[:], rhs=b2_sb[l][:],
                                     start=False, stop=True)
                    h_store(row0 + r, hp)

            # ---------------- phase B: layer 1 + chunked AllGather ----------
            def store_h1(rt, hp):
                hs = h1wp.tile([128, 256], H1DT, tag="h1")
                nc.scalar.activation(out=hs[:], in_=hp[:], func=AF.Relu)
                eng = nc.sync if rt % 2 == 0 else nc.scalar
                eng.dma_start(
                    out=h1loc[rt * 128:(rt + 1) * 128, :], in_=hs[:])

            NCHK = NTILE // 4
            ag_after = {}
            for cgi in range(NCHUNK):
                ag_after[-(-CH * (cgi + 1) // 512) - 1] = cgi
            for ch in range(NCHK):
                cnt_sb = work.tile([19, 512], f16, tag="cnt")
                nc.sync.dma_start(out=cnt_sb[:],
                                  in_=cnt19[:, ch * 512:(ch + 1) * 512])
                agg_sb = []
                for k in range(2):
                    ap_ = ps512.tile([128, 512], f32, space="PSUM",
                                     tag="ps512")
                    nc.tensor.matmul(
                        ap_[:], lhsT=tab1_sb[:, k * 128:(k + 1) * 128],
                        rhs=cnt_sb[:], start=True, stop=True)
                    asb = aggp.tile([128, 512], f16, tag="agg")
                    nc.vector.tensor_copy(out=asb[:], in_=ap_[:])
                    agg_sb.append(asb)
                mlp(0, agg_sb, ch * 4, store_h1)
                if ch in ag_after:
                    cgi = ag_after[ch]
                    nc.gpsimd.collective_compute(
                        "AllGather", OP.bypass,
                        replica_groups=[list(range(NCORE))],
                        ins=[h1loc[cgi * CH:(cgi + 1) * CH, :].opt()],
                        outs=[h1full[cgi][:].opt()])

            # ---------------- phase C: layer 2 ------------------------------
            h2_tiles = []
            pool_done = [0]  # blocks pooled so far

            def store_h2(rt, hp):
                hs = h2p.tile([128, 256], f16, tag="h2")
                nc.scalar.activation(out=hs[:], in_=hp[:], func=AF.Relu)
                h2_tiles.append(hs)

            def pool_block(b):
                pp_ = psp.tile([128, 256], f32, space="PSUM", tag="pp",
                               name="pp_")
                pps = [pp_[:, 0:128], pp_[:, 128:256]]
                ops = []
                for s in range(S):
                    nt = b * S + s
                    op_ = ohp.tile([128, 128], f16, tag="ohp")
                    nc.vector.tensor_scalar(
                        out=op_[:], in0=colidx_sb[:],
                        scalar1=indloc_sb[:, nt:nt + 1], scalar2=None,
                        op0=OP.is_equal)
                    ops.append(op_)
                for k in range(2):
                    for s in range(S):
                        nt = b * S + s
                        nc.tensor.matmul(
                            pps[k],
                            lhsT=h2_tiles[nt][:, k * 128:(k + 1) * 128],
                            rhs=ops[s][:], start=(s == 0), stop=(s == S - 1))
                    nc.vector.tensor_copy(
                        out=cat_sb[2 + k][:, b * 128:(b + 1) * 128],
                        in_=pps[k])

            agg2_box = [None]

            def gather_gb(gb):
                gout = []
                for cls in range(NCHUNK):
                    nidx = iw_len[gb][cls]
                    gtiles = nidx // 128
                    if gtiles == 0:
                        gout.append(None)
                        continue
                    gt = goutp.tile([128, meta["GMAX"], 256], H1DT,
                                    tag=f"g{cls}", name=f"gout{cls}")
                    nc.gpsimd.dma_gather(
                        out_ap=gt[:, 0:gtiles, :],
                        in_ap=h1full[cls][:, :],
                        idxs_ap=gidx_sb[:, iw_off[gb][cls]:
                                        iw_off[gb][cls] + nidx // 16],
                        num_idxs=nidx,
                        num_idxs_reg=nidx,
                        elem_size=256,
                    )
                    gout.append(gt)
                return gout

            def make_ohs(nt):
                ohs = []
                for cls in range(NCHUNK):
                    for j in range(Tmax[nt][cls]):
                        tg = tile_off[nt][cls] + j
                        oh_sb = ohp.tile([128, 128], H1DT, tag="oh")
                        nc.vector.tensor_scalar(
                            out=oh_sb[:], in0=colidx_sb[:],
                            scalar1=dstl_sb[:, tg:tg + 1], scalar2=None,
                            op0=OP.is_equal)
                        ohs.append(oh_sb)
                return ohs

            def scatter_nt_k(gb, gout, nt, gps, ntiles, ohs, k):
                ti = 0
                for cls in range(NCHUNK):
                    # group offset of this nt inside gout[cls]
                    goff = sum(Tmax[nt2][cls]
                               for nt2 in range(gb * GB_NT, nt))
                    for j in range(Tmax[nt][cls]):
                        oh_sb = ohs[ti]
                        ti += 1
                        nc.tensor.matmul(
                            gps[k],
                            lhsT=gout[cls][:, goff + j,
                                           k * 128:(k + 1) * 128],
                            rhs=oh_sb[:], start=False, stop=(ti == ntiles))

            def do_nt(gb, gout, nt):
                if nt % 4 == 0:
                    agg2_box[0] = [aggp.tile([128, 512], f16, tag="agg",
                                             name=f"agg2_{k}")
                                   for k in range(2)]
                agg2_sb = agg2_box[0]
                gp_ = psg.tile([128, 256], f32, space="PSUM", tag="gp",
                               name="gp_")
                gps = [gp_[:, 0:128], gp_[:, 128:256]]
                ntiles = sum(Tmax[nt][cls] for cls in range(NCHUNK))
                assert ntiles > 0
                ohs = make_ohs(nt)
                for k in range(2):
                    nc.tensor.matmul(
                        gps[k], lhsT=ee2_sb[:, k * 128:(k + 1) * 128],
                        rhs=cnt10_sb[:, nt * 128:(nt + 1) * 128],
                        start=True, stop=False)
                    scatter_nt_k(gb, gout, nt, gps, ntiles, ohs, k)
                    nc.vector.tensor_copy(
                        out=agg2_sb[k][:, (nt % 4) * 128:(nt % 4 + 1) * 128],
                        in_=gps[k])
                if nt % 4 == 3:
                    ch = nt // 4
                    if DBG_TAPS:
                        for k in range(2):
                            nc.sync.dma_start(
                                out=dbg_agg2[k][:, ch * 512:(ch + 1) * 512],
                                in_=agg2_sb[k][:])
                    mlp(1, agg2_sb, ch * 4, store_h2)
                    while ((pool_done[0] + 1) * S - 1) // 4 <= ch:
                        pool_block(pool_done[0])
                        pool_done[0] += 1

            for gb in range(NGB):
                gout = gather_gb(gb)
                for nt in range(gb * GB_NT, (gb + 1) * GB_NT):
                    do_nt(gb, gout, nt)
            assert pool_done[0] == NBLK, pool_done

            # ---------------- phase D: BN1 -> proj -> BN2 -> out ------------
            if DBG_TAPS:
                for t in range(4):
                    nc.sync.dma_start(out=dbg_cat[t], in_=cat_sb[t][:])
                nc.sync.dma_start(out=dbg_h1[:, :], in_=h1loc[:, :])
            for t in range(2):
                tile_stats(cat_sb[2 + t], st1loc, 2, t)
            nc.gpsimd.collective_compute(
                "AllReduce", OP.add, replica_groups=[list(range(NCORE))],
                ins=[st1loc[:].opt()], outs=[st1glob[:].opt()])

            def scale_bias(g_sb, gam, bet):
                mu = work.tile([128, 1], f32, tag="mu")
                nc.vector.tensor_scalar_mul(mu[:], g_sb[:, 0:1], 1.0 / N)
                var = work.tile([128, 1], f32, tag="var")
                nc.vector.tensor_scalar_mul(var[:], g_sb[:, 1:2], 1.0 / N)
                musq = work.tile([128, 1], f32, tag="musq")
                nc.vector.tensor_tensor(out=musq[:], in0=mu[:], in1=mu[:],
                                        op=OP.mult)
                nc.vector.tensor_tensor(out=var[:], in0=var[:], in1=musq[:],
                                        op=OP.subtract)
                sd = work.tile([128, 1], f32, tag="sd")
                nc.scalar.activation(out=sd[:], in_=var[:], func=AF.Sqrt,
                                     bias=eps_sb[:, 0:1])
                rstd = work.tile([128, 1], f32, tag="rstd")
                nc.vector.reciprocal(rstd[:], sd[:])
                sc = work.tile([128, 1], f32, tag="sc")
                nc.vector.tensor_tensor(out=sc[:], in0=rstd[:], in1=gam[:],
                                        op=OP.mult)
                bi = work.tile([128, 1], f32, tag="bi")
                nc.vector.tensor_tensor(out=bi[:], in0=mu[:], in1=sc[:],
                                        op=OP.mult)
                nc.vector.tensor_tensor(out=bi[:], in0=bet[:], in1=bi[:],
                                        op=OP.subtract)
                return sc, bi

            bi_sb = []
            for t in range(4):
                g_sb = statp.tile([128, 2], f32, tag="gstat")
                nc.sync.dma_start(out=g_sb[:],
                                  in_=st1glob[t * 128:(t + 1) * 128, :])
                sc, bi = scale_bias(g_sb, bng_sb[t], bnb_sb[t])
                nc.vector.tensor_scalar(
                    out=cat_sb[t][:], in0=cat_sb[t][:], scalar1=sc[:, 0:1],
                    scalar2=bi[:, 0:1], op0=OP.mult, op1=OP.add)
                bi_sb.append(bi)

            out2_sb = [wide.tile([128, CPAD], f32, tag=f"o2_{c2}",
                                 name=f"o2sb{c2}") for c2 in range(2)]
            for w in range(CPAD // 512):
                for c2 in range(2):
                    pp = ps512.tile([128, 512], f32, space="PSUM",
                                    tag="ps512")
                    for k in range(4):
                        nc.tensor.matmul(
                            pp[:], lhsT=wp_sb[k][:, c2 * 128:(c2 + 1) * 128],
                            rhs=cat_sb[k][:, w * 512:(w + 1) * 512],
                            start=(k == 0), stop=(k == 3))
                    nc.vector.tensor_scalar(
                        out=out2_sb[c2][:, w * 512:(w + 1) * 512], in0=pp[:],
                        scalar1=bp_sb[c2][:, 0:1], scalar2=None, op0=OP.add)

            # v[c2] = proj of the all-pad (zero cat) column, for stats fixup
            v_sb = []
            for c2 in range(2):
                vp_ = ps512.tile([128, 512], f32, space="PSUM", tag="ps512")
                vp = vp_[:, 0:1]
                for k in range(4):
                    nc.tensor.matmul(
                        vp, lhsT=wp_sb[k][:, c2 * 128:(c2 + 1) * 128],
                        rhs=bi_sb[k][:], start=(k == 0), stop=(k == 3))
                v_ = work.tile([128, 1], f32, tag="v")
                nc.vector.tensor_tensor(out=v_[:], in0=vp,
                                        in1=bp_sb[c2][:], op=OP.add)
                v_sb.append(v_)

            for c2 in range(2):
                s_sb = statp.tile([128, 2], f32, tag="stat2")
                nc.vector.tensor_reduce(
                    out=s_sb[:, 0:1], in_=out2_sb[c2][:],
                    axis=mybir.AxisListType.X, op=OP.add)
                s1t = statp.tile([128, 1], f32, tag="s1u")
                for w in range(CPAD // 512):
                    nc.vector.tensor_tensor(
                        out=sqtmp[:], in0=out2_sb[c2][:, w * 512:(w + 1) * 512],
                        in1=out2_sb[c2][:, w * 512:(w + 1) * 512], op=OP.mult)
                    if w == 0:
                        nc.vector.tensor_reduce(
                            out=s_sb[:, 1:2], in_=sqtmp[:],
                            axis=mybir.AxisListType.X, op=OP.add)
                    else:
                        nc.vector.tensor_reduce(
                            out=s1t[:], in_=sqtmp[:],
                            axis=mybir.AxisListType.X, op=OP.add)
                        nc.vector.tensor_tensor(
                            out=s_sb[:, 1:2], in0=s_sb[:, 1:2], in1=s1t[:],
                            op=OP.add)
                # remove the NPAD fake columns (each equals v)
                vs = work.tile([128, 1], f32, tag="vs")
                nc.vector.tensor_scalar_mul(vs[:], v_sb[c2][:], float(NPAD))
                nc.vector.tensor_tensor(out=s_sb[:, 0:1], in0=s_sb[:, 0:1],
                                        in1=vs[:], op=OP.subtract)
                vs2 = work.tile([128, 1], f32, tag="vs2")
                nc.vector.tensor_tensor(out=vs2[:], in0=v_sb[c2][:],
                                        in1=vs[:], op=OP.mult)
                nc.vector.tensor_tensor(out=s_sb[:, 1:2], in0=s_sb[:, 1:2],
                                        in1=vs2[:], op=OP.subtract)
                nc.sync.dma_start(out=st2loc[c2 * 128:(c2 + 1) * 128, :],
                                  in_=s_sb[:])
            nc.gpsimd.collective_compute(
                "AllReduce", OP.add, replica_groups=[list(range(NCORE))],
                ins=[st2loc[:].opt()], outs=[st2glob[:].opt()])
            for c2 in range(2):
                g_sb = statp.tile([128, 2], f32, tag="gstat2")
                nc.sync.dma_start(out=g_sb[:],
                                  in_=st2glob[c2 * 128:(c2 + 1) * 128, :])
                sc, bi = scale_bias(g_sb, ng_sb[c2], nb_sb[c2])
                nc.vector.tensor_scalar(
                    out=out2_sb[c2][:], in0=out2_sb[c2][:],
                    scalar1=sc[:, 0:1], scalar2=bi[:, 0:1],
                    op0=OP.mult, op1=OP.add)
                eng = nc.sync if c2 == 0 else nc.scalar
                eng.dma_start(out=out[c2], in_=out2_sb[c2][:])

    nc.compile()
    return nc


_CACHE = {}


def kernel(**inputs):
    from concourse.bass_utils import run_bass_kernel_spmd

    per_core, meta, colmap = _preprocess(inputs)
    wm = _weight_maps(inputs)

    key = (meta["ROWS"], meta["T_E"],
           tuple(tuple(r) for r in meta["Tmax"]))
    if key not in _CACHE:
        _CACHE[key] = _build(meta)
    nc = _CACHE[key]

    in_maps = []
    for c in range(NCORE):
        m = dict(per_core[c])
        m.update(wm)
        in_maps.append(m)

    trace = bool(int(os.environ.get("KERNEL_TRACE", "0")))
    res = run_bass_kernel_spmd(nc, in_maps, list(range(NCORE)), trace=trace)
    kernel.last_results = res

    out_np = np.empty((N, D), np.float32)
    for c in range(NCORE):
        arr = res.results[c]["out"]  # [2, 128, CPAD]
        cols = np.flatnonzero(colmap[c] >= 0)
        ids = colmap[c][cols]
        out_np[ids, :128] = arr[0][:, cols].T
        out_np[ids, 128:] = arr[1][:, cols].T
    return out_np


# revision 32
# speedup vs baseline: 2.0416x; 1.1430x over previous
"""Trainium2 Bass kernel for the GNN k-hop subgraph encoder (GIN, L=2, D=256).

Strategy (8 cores, graph-parallel, v2):
  - Host: sort subgraph nodes by indicator (center id); shard by center
    (2500 centers/core); bin-pack centers into 20 blocks of <=128 centers
    and <=S*128 slots (S=5 typically) -> ROWS = 20*S*128 tight node rows.
  - Layer 1 needs NO gather: layer-1 aggregation is counts[19, ROWS]^T @
    table[19, 256] (counts built on host), then the GIN MLP on device.
  - h1 is stored fp8 (e4m3) and AllGather'd in NCHUNK=4 chunks overlapped
    with layer-1 compute; layer 2 gathers h1[src] rows with dma_gather
    (thousands of rows per instruction, 4 address-region classes to fit
    int16 indices), scatter-adds via one-hot matmuls in PSUM where the
    one-hots are generated on-device (iota/is_equal on the vector engine).
    Self-loops skip the gather entirely: identity matmul from the fp16 h1
    kept in SBUF.
  - Pooling onto centers is an incremental one-hot matmul per block;
    BatchNorm stats via one AllReduce (origin-half stats computed early);
    projection + final norm on device; output stays feature-major and the
    host transposes/un-permutes.
All matmul operands fp16/fp8, accumulation fp32 in PSUM.
"""
import os
import sys

import numpy as np

sys.path.insert(0, "/opt/trn_rl_repo")

N = 20000
NSUB = 100000
ESUB = 300000
D = 256
EPS = 1e-5
NCORE = 8
CPC = N // NCORE            # 2500 centers per core
NBLK = 20                   # center blocks of 128
CPAD = NBLK * 128           # 2560
NCHUNK = 4                  # AllGather chunks == gather region classes
GB_NT = 4                   # node tiles per gather block
H1_FP8 = False
DBG_TAPS = False


# ----------------------------------------------------------------------------
# host preprocessing
# ----------------------------------------------------------------------------
def _pack_centers(counts, cap_slots):
    """Greedy least-loaded packing of centers into NBLK blocks.
    counts: [CPC] slots per center. Returns blocks: list of NBLK lists of
    center ids, or None if infeasible under (cap_slots, 128 centers)."""
    order = np.argsort(-counts, kind="stable")
    loads = np.zeros(NBLK, np.int64)
    ncent = np.zeros(NBLK, np.int64)
    blocks = [[] for _ in range(NBLK)]
    for ctr in order:
        k = counts[ctr]
        # least-loaded block with room
        best, bestload = -1, None
        for b in range(NBLK):
            if ncent[b] < 128 and loads[b] + k <= cap_slots:
                if bestload is None or loads[b] < bestload:
                    best, bestload = b, loads[b]
        if best < 0:
            return None
        blocks[best].append(int(ctr))
        loads[best] += k
        ncent[best] += 1
    return blocks


def _preprocess(inputs):
    x = np.asarray(inputs["x"], np.int64)
    sni = np.asarray(inputs["subgraph_node_index"], np.int64)
    sei = np.asarray(inputs["subgraph_edge_index"], np.int64)
    sea = np.asarray(inputs["subgraph_edge_attr"], np.int64)
    sii = np.asarray(inputs["subgraph_indicator_index"], np.int64)

    pi = np.argsort(sii, kind="stable")
    ind_s = sii[pi]
    node_s = sni[pi]
    sub_lo = np.searchsorted(ind_s, np.arange(0, N + 1, CPC))

    # --- per-center slot counts, bin-pack into blocks -----------------------
    ctr_cnt = np.zeros(N, np.int64)
    np.add.at(ctr_cnt, ind_s, 1)
    ctr_cnt = ctr_cnt.reshape(NCORE, CPC)

    S = 5
    packs = []
    for c in range(NCORE):
        blocks = _pack_centers(ctr_cnt[c], S * 128)
        if blocks is None:
            S = 6
            packs = []
            for c2 in range(NCORE):
                blocks = _pack_centers(ctr_cnt[c2], S * 128)
                assert blocks is not None, "center packing failed at S=6"
                packs.append(blocks)
            break
        packs.append(blocks)
    ROWS = NBLK * S * 128
    NTILE = ROWS // 128
    CH = ROWS // NCHUNK
    REG = NCORE * CH
    assert REG <= 32767 and ROWS % 512 == 0

    # --- slot assignment ----------------------------------------------------
    # pos -> (core, slot); slot layout: block b owns [b*S*128, (b+1)*S*128)
    slot_of_pos = np.zeros(NSUB, np.int64)
    indloc = np.full((NCORE, ROWS), -1, np.int64)   # slot -> center-local col
    colmap = np.full((NCORE, CPAD), -1, np.int64)   # cat col -> global center
    ctr_start = np.zeros(N + 1, np.int64)           # run start of each center
    np.cumsum(ctr_cnt.reshape(-1), out=ctr_start[1:])
    for c in range(NCORE):
        for b in range(NBLK):
            off = b * S * 128
            for j, ctr in enumerate(packs[c][b]):
                g = c * CPC + ctr
                lo, hi = ctr_start[g], ctr_start[g + 1]
                k = hi - lo
                slot_of_pos[pi[lo:hi]] = off + np.arange(k)
                indloc[c, off:off + k] = j
                colmap[c, b * 128 + j] = g
                off += k
    cps = np.searchsorted(sub_lo, np.arange(NSUB), side="right") - 1
    core_of_pos = np.empty(NSUB, np.int64)
    core_of_pos[pi] = cps

    # --- L1 count matrix (includes self-loops) ------------------------------
    ntype = x[node_s, 0] * 3 + x[node_s, 1]
    # re-map to per-position arrays in original position index space
    ntype_pos = np.zeros(NSUB, np.int64)
    ntype_pos[pi] = ntype
    src = sei[0]
    dst = sei[1]
    ecombo = sea[:, 0] * 3 + sea[:, 1]

    cnt19 = np.zeros((NCORE, 19, ROWS), np.float32)
    dcore = core_of_pos[dst]
    dslot = slot_of_pos[dst]
    np.add.at(cnt19, (dcore, ntype_pos[src], dslot), 1.0)
    np.add.at(cnt19, (dcore, 9 + ecombo, dslot), 1.0)
    # self loops: type of self + combo 9
    score = core_of_pos[np.arange(NSUB)]
    sslot = slot_of_pos[np.arange(NSUB)]
    np.add.at(cnt19, (score, ntype_pos[np.arange(NSUB)], sslot), 1.0)
    np.add.at(cnt19, (score, np.full(NSUB, 18), sslot), 1.0)

    # --- gather address of each position ------------------------------------
    # h1full layout: [chunk][core][CH rows]
    gaddr_chunk = slot_of_pos // CH
    gaddr_idx = core_of_pos * CH + slot_of_pos % CH   # index within region

    # --- edge tiling: per (nt, cls) lists, shared Tmax ----------------------
    # (self-loops are handled by identity matmuls on device, not gathered)
    dcore2 = core_of_pos[dst]
    dslot2 = slot_of_pos[dst]
    scls = gaddr_chunk[src]
    sidx = gaddr_idx[src]
    dnt = dslot2 // 128
    dloc = dslot2 % 128
    ecnt = np.zeros((NCORE, NTILE, NCHUNK), np.int64)
    np.add.at(ecnt, (dcore2, dnt, scls), 1)
    Tmax = np.ceil(ecnt.max(axis=0) / 128).astype(np.int64)  # [NTILE, NCHUNK]

    NGB = NTILE // GB_NT
    assert NTILE % GB_NT == 0
    T_E = int(Tmax.sum())
    # tile/program ordering: gb -> cls -> nt in gb -> tile
    tile_off = np.zeros((NTILE, NCHUNK), np.int64)  # global tile id of first
    iw_off = np.zeros((NGB, NCHUNK), np.int64)      # idx col offset per instr
    iw_len = np.zeros((NGB, NCHUNK), np.int64)      # num_idxs per instr
    t0 = 0
    col0 = 0
    for gb in range(NGB):
        for cls in range(NCHUNK):
            n_idx = 0
            for nt in range(gb * GB_NT, (gb + 1) * GB_NT):
                tile_off[nt, cls] = t0
                t0 += Tmax[nt, cls]
                n_idx += int(Tmax[nt, cls]) * 128
            iw_off[gb, cls] = col0
            iw_len[gb, cls] = n_idx
            col0 += n_idx // 16
    assert t0 == T_E
    IDXCOLS = col0

    per_core = []
    for c in range(NCORE):
        em = dcore2 == c
        es_cls, es_idx = scls[em], sidx[em]
        es_nt, es_loc = dnt[em], dloc[em]
        order = np.argsort(es_nt * NCHUNK + es_cls, kind="stable")
        es_cls, es_idx = es_cls[order], es_idx[order]
        es_nt, es_loc = es_nt[order], es_loc[order]
        bounds = np.searchsorted(
            es_nt * NCHUNK + es_cls, np.arange(NTILE * NCHUNK + 1))

        gidx16 = np.zeros((128, IDXCOLS), np.int16)
        dstl = np.full((T_E, 128), -1.0, np.float32)
        for nt in range(NTILE):
            for cls in range(NCHUNK):
                a, b = bounds[nt * NCHUNK + cls], bounds[nt * NCHUNK + cls + 1]
                k = b - a
                cap = int(Tmax[nt, cls]) * 128
                assert k <= cap, (c, nt, cls, k, cap)
                vals = np.zeros(cap, np.int16)
                vals[:k] = es_idx[a:b]
                tg = tile_off[nt, cls]
                dstl[tg:tg + cap // 128] = -1.0
                dl = np.full(cap, -1.0, np.float32)
                dl[:k] = es_loc[a:b]
                dstl[tg:tg + cap // 128] = dl.reshape(-1, 128)
                # idx columns: within instruction (gb, cls)
                gb = nt // GB_NT
                # offset of this nt's idxs inside the instruction
                pre = int((Tmax[nt // GB_NT * GB_NT:nt, cls]).sum()) * 128
                cstart = int(iw_off[gb, cls]) + pre // 16
                arr16 = vals.reshape(-1, 16).T  # [16, cap/16]
                gidx16[:, cstart:cstart + cap // 16] = np.tile(arr16, (8, 1))

        oh9 = np.zeros((9, CPAD), np.float32)
        cols = np.flatnonzero(colmap[c] >= 0)
        gctr = colmap[c][cols]
        oh9[x[gctr, 0] * 3 + x[gctr, 1], cols] = 1.0

        il = np.zeros((128, NTILE), np.float32)
        il[:, :] = indloc[c].reshape(NTILE, 128).T
        oh_e = np.zeros((T_E, 128, 128), np.float16)
        tg_, e_ = np.nonzero(dstl >= 0)
        oh_e[tg_, e_, dstl[tg_, e_].astype(np.int64)] = 1.0
        per_core.append(dict(
            cnt19=cnt19[c].astype(np.float16),
            cnt10=cnt19[c, 9:19].astype(np.float16),
            gidx16=gidx16,
            oh_e=oh_e,
            indloc=il.astype(np.float32),
            oh9=oh9.astype(np.float16),
        ))

    GMAX = 0
    for gb in range(NGB):
        for cls in range(NCHUNK):
            GMAX = max(GMAX, int(iw_len[gb, cls]) // 128)
    meta = dict(S=S, ROWS=ROWS, NTILE=NTILE, CH=CH, REG=REG, NGB=NGB,
                T_E=T_E, IDXCOLS=IDXCOLS, GMAX=GMAX,
                Tmax=[[int(v) for v in row] for row in Tmax],
                tile_off=[[int(v) for v in row] for row in tile_off],
                iw_off=[[int(v) for v in row] for row in iw_off],
                iw_len=[[int(v) for v in row] for row in iw_len])
    return per_core, meta, colmap


def _weight_maps(inputs):
    f16 = np.float16
    f32 = np.float32
    emb1 = np.asarray(inputs["emb1"], f32)
    emb2 = np.asarray(inputs["emb2"], f32)
    ee1 = np.asarray(inputs["edge_e1"], f32)
    ee2 = np.asarray(inputs["edge_e2"], f32)

    # TAB1[t] = emb1[t//3]+emb2[t%3] (t<9); TAB1[9+u] = ee1[0][b]+ee2[0][d]
    # (u<9: b=u//3, d=u%3; u=9: self loop b=4, d=0)
    tab1 = np.zeros((19, D), f32)
    for t in range(9):
        tab1[t] = emb1[t // 3] + emb2[t % 3]
    for u in range(9):
        tab1[9 + u] = ee1[0][u // 3] + ee2[0][u % 3]
    tab1[18] = ee1[0][4] + ee2[0][0]
    ee2t = np.zeros((10, D), f32)
    for u in range(9):
        ee2t[u] = ee1[1][u // 3] + ee2[1][u % 3]
    ee2t[9] = ee1[1][4] + ee2[1][0]

    colidx = np.tile(np.arange(128, dtype=f32), (128, 1))

    return dict(
        tab1=tab1.astype(f16), ee2t=ee2t.astype(f16),
        colidx=colidx.astype(f16),
        w1=np.asarray(inputs["W1"], f32).astype(f16),
        w2=np.asarray(inputs["W2"], f32).astype(f16),
        b1t=np.asarray(inputs["b1"], f32).reshape(2, 4, 128, 1),
        b2f=np.asarray(inputs["b2"], f32).reshape(2, 1, 256).astype(f16),
        wp=np.asarray(inputs["Wp"], f32),
        bpt=np.asarray(inputs["bp"], f32).reshape(2, 128, 1),
        bngt=np.asarray(inputs["bn_cat_g"], f32).reshape(4, 128, 1),
        bnbt=np.asarray(inputs["bn_cat_b"], f32).reshape(4, 128, 1),
        ngt=np.asarray(inputs["norm_g"], f32).reshape(2, 128, 1),
        nbt=np.asarray(inputs["norm_b"], f32).reshape(2, 128, 1),
    )


# ----------------------------------------------------------------------------
# bass kernel
# ----------------------------------------------------------------------------
def _build(meta):
    from concourse import bass, bacc, mybir, tile
    from concourse.masks import make_identity

    f16 = mybir.dt.float16
    f32 = mybir.dt.float32
    f8 = mybir.dt.float8e4
    i16 = mybir.dt.int16
    AF = mybir.ActivationFunctionType
    OP = mybir.AluOpType

    H1DT = f8 if H1_FP8 else f16
    H1SZ = 1 if H1_FP8 else 2

    S = meta["S"]
    ROWS = meta["ROWS"]
    NTILE = meta["NTILE"]
    CH = meta["CH"]
    REG = meta["REG"]
    NGB = meta["NGB"]
    T_E = meta["T_E"]
    IDXCOLS = meta["IDXCOLS"]
    Tmax = meta["Tmax"]
    tile_off = meta["tile_off"]
    iw_off = meta["iw_off"]
    iw_len = meta["iw_len"]
    NPAD = CPAD - CPC  # padded (fake) center columns per core

    nc = bacc.Bacc("TRN2", target_bir_lowering=False, debug=False,
                   num_devices=NCORE)

    def din(name, shape, dt):
        return nc.dram_tensor(name, shape, dt, kind="ExternalInput")

    cnt19 = din("cnt19", [19, ROWS], f16)
    cnt10 = din("cnt10", [10, ROWS], f16)
    gidx16 = din("gidx16", [128, IDXCOLS], i16)
    oh_e = din("oh_e", [T_E, 128, 128], f16)
    indloc = din("indloc", [128, NTILE], f32)
    oh9 = din("oh9", [9, CPAD], f16)
    tab1 = din("tab1", [19, 256], f16)
    ee2t = din("ee2t", [10, 256], f16)
    colidx = din("colidx", [128, 128], f16)
    w1 = din("w1", [2, 256, 512], f16)
    w2 = din("w2", [2, 512, 256], f16)
    b1t = din("b1t", [2, 4, 128, 1], f32)
    b2f = din("b2f", [2, 1, 256], f16)
    wp = din("wp", [512, 256], f32)
    bpt = din("bpt", [2, 128, 1], f32)
    bngt = din("bngt", [4, 128, 1], f32)
    bnbt = din("bnbt", [4, 128, 1], f32)
    ngt = din("ngt", [2, 128, 1], f32)
    nbt = din("nbt", [2, 128, 1], f32)
    out = nc.dram_tensor("out", [2, 128, CPAD], f16, kind="ExternalOutput")
    if DBG_TAPS:
        dbg_agg2 = nc.dram_tensor("dbg_agg2", [2, 128, ROWS], f16,
                                  kind="ExternalOutput")
        dbg_cat = nc.dram_tensor("dbg_cat", [4, 128, CPAD], f32,
                                 kind="ExternalOutput")
        dbg_h1 = nc.dram_tensor("dbg_h1", [ROWS, 256], f16,
                                kind="ExternalOutput")

    with tile.TileContext(nc) as tc:
        with (
            tc.tile_pool(name="const", bufs=1) as cpool,
            tc.tile_pool(name="wide", bufs=1) as wide,
            tc.tile_pool(name="work", bufs=4) as work,
            tc.tile_pool(name="aggp", bufs=3) as aggp,
            tc.tile_pool(name="mids", bufs=6) as midp,
            tc.tile_pool(name="statp", bufs=8) as statp,
            tc.tile_pool(name="ohs", bufs=14) as ohp,
            tc.tile_pool(name="h1w", bufs=4) as h1wp,
            tc.tile_pool(name="h2s", bufs=4 + 2 * S) as h2p,
            tc.tile_pool(name="gout", bufs=2) as goutp,
            tc.tile_pool(name="selfp", bufs=6) as selfp,
            tc.tile_pool(name="ps512", bufs=3, space="PSUM") as ps512,
            tc.tile_pool(name="ps256", bufs=2, space="PSUM") as ps256,
            tc.tile_pool(name="psg", bufs=2, space="PSUM") as psg,
            tc.tile_pool(name="psp", bufs=1, space="PSUM") as psp,
            tc.tile_pool(name="dram", bufs=1, space="DRAM") as dram,
        ):
            # ---------------- constants / weights into SBUF ----------------
            _ldc = [0]

            def load(pool, eng, src, shape, dt):
                _ldc[0] += 1
                nm = f"ld{_ldc[0]}"
                t = pool.tile(shape, dt, name=nm, tag=nm)
                eng.dma_start(out=t[:], in_=src)
                return t

            cnt10_sb = load(cpool, nc.sync, cnt10[:, :], [10, ROWS], f16)
            gidx_sb = load(cpool, nc.scalar, gidx16[:, :], [128, IDXCOLS], i16)
            indloc_sb = load(cpool, nc.scalar, indloc[:, :], [128, NTILE], f32)
            oh9_sb = load(cpool, nc.scalar, oh9[:, :], [9, CPAD], f16)
            tab1_sb = load(cpool, nc.scalar, tab1[:, :], [19, 256], f16)
            ee2_sb = load(cpool, nc.scalar, ee2t[:, :], [10, 256], f16)
            colidx_sb = load(cpool, nc.scalar, colidx[:, :], [128, 128], f16)
            w1_sb = [[load(cpool, nc.scalar, w1[l, k * 128:(k + 1) * 128, :],
                           [128, 512], f16) for k in range(2)]
                     for l in range(2)]
            w2_sb = [[load(cpool, nc.scalar, w2[l, k * 128:(k + 1) * 128, :],
                           [128, 256], f16) for k in range(4)]
                     for l in range(2)]
            wp_sb = [load(cpool, nc.scalar, wp[k * 128:(k + 1) * 128, :],
                          [128, 256], f32) for k in range(4)]
            b1_sb = [[load(cpool, nc.scalar, b1t[l, m], [128, 1], f32)
                      for m in range(4)] for l in range(2)]
            b2_sb = [load(cpool, nc.scalar, b2f[l], [1, 256], f16)
                     for l in range(2)]
            bp_sb = [load(cpool, nc.scalar, bpt[c2], [128, 1], f32)
                     for c2 in range(2)]
            bng_sb = [load(cpool, nc.scalar, bngt[t], [128, 1], f32)
                      for t in range(4)]
            bnb_sb = [load(cpool, nc.scalar, bnbt[t], [128, 1], f32)
                      for t in range(4)]
            ng_sb = [load(cpool, nc.scalar, ngt[t], [128, 1], f32)
                     for t in range(2)]
            nb_sb = [load(cpool, nc.scalar, nbt[t], [128, 1], f32)
                     for t in range(2)]

            ones_sb = cpool.tile([1, 128], f16)
            nc.vector.memset(ones_sb[:], 1.0)
            eps_sb = cpool.tile([128, 1], f32)
            nc.vector.memset(eps_sb[:], EPS)
            ident = cpool.tile([128, 128], f16)
            make_identity(nc, ident[:])

            # DRAM bounces
            h1loc = dram.tile([ROWS, 256], H1DT)
            h1full = [dram.tile([NCORE * CH, 256], H1DT,
                                addr_space="Shared", name=f"h1full{cgi}",
                                tag=f"h1full{cgi}")
                      for cgi in range(NCHUNK)]
            st1loc = dram.tile([512, 2], f32)
            st1glob = dram.tile([512, 2], f32)
            st2loc = dram.tile([256, 2], f32)
            st2glob = dram.tile([256, 2], f32)

            # ---------------- phase A: origin half of cat + stats ----------
            cat_sb = [wide.tile([128, CPAD], f32, tag=f"cat{t}",
                                name=f"cat{t}") for t in range(4)]
            for k in range(2):
                for w in range(CPAD // 512):
                    op_ = ps512.tile([128, 512], f32, space="PSUM",
                                     tag="ps512")
                    nc.tensor.matmul(
                        op_[:], lhsT=tab1_sb[0:9, k * 128:(k + 1) * 128],
                        rhs=oh9_sb[:, w * 512:(w + 1) * 512],
                        start=True, stop=True)
                    nc.vector.tensor_copy(
                        out=cat_sb[k][:, w * 512:(w + 1) * 512], in_=op_[:])

            sqtmp = wide.tile([128, 512], f32, tag="sqtmp", name="sqtmp")

            def tile_stats(src_sb, loc, row0, t):
                s_sb = statp.tile([128, 2], f32, tag="stat")
                nc.vector.tensor_reduce(
                    out=s_sb[:, 0:1], in_=src_sb[:],
                    axis=mybir.AxisListType.X, op=OP.add)
                s1t = statp.tile([128, 1], f32, tag="s1t")
                for w in range(CPAD // 512):
                    nc.vector.tensor_tensor(
                        out=sqtmp[:], in0=src_sb[:, w * 512:(w + 1) * 512],
                        in1=src_sb[:, w * 512:(w + 1) * 512], op=OP.mult)
                    if w == 0:
                        nc.vector.tensor_reduce(
                            out=s_sb[:, 1:2], in_=sqtmp[:],
                            axis=mybir.AxisListType.X, op=OP.add)
                    else:
                        nc.vector.tensor_reduce(
                            out=s1t[:], in_=sqtmp[:],
                            axis=mybir.AxisListType.X, op=OP.add)
                        nc.vector.tensor_tensor(
                            out=s_sb[:, 1:2], in0=s_sb[:, 1:2], in1=s1t[:],
                            op=OP.add)
                nc.sync.dma_start(
                    out=loc[(row0 + t) * 128:(row0 + t + 1) * 128, :],
                    in_=s_sb[:])

            for t in range(2):
                tile_stats(cat_sb[t], st1loc, 0, t)

            # ---------------- shared MLP block (512 rows) -------------------
            def mlp(l, agg_sb, row0, h_store):
                mid_sb = []
                for m in range(4):
                    mp = ps512.tile([128, 512], f32, space="PSUM", tag="ps512")
                    for k in range(2):
                        nc.tensor.matmul(
                            mp[:],
                            lhsT=w1_sb[l][k][:, m * 128:(m + 1) * 128],
                            rhs=agg_sb[k][:], start=(k == 0), stop=(k == 1))
                    ms = midp.tile([128, 512], f16, tag="mid")
                    nc.scalar.activation(out=ms[:], in_=mp[:], func=AF.Relu,
                                         bias=b1_sb[l][m][:])
                    mid_sb.append(ms)
                for r in range(4):
                    hp = ps256.tile([128, 256], f32, space="PSUM", tag="ps256")
                    for k in range(4):
                        nc.tensor.matmul(
                            hp[:], lhsT=mid_sb[k][:, r * 128:(r + 1) * 128],
                            rhs=w2_sb[l][k][:], start=(k == 0), stop=False)
                    nc.tensor.matmul(hp[:], lhsT=ones_sb[:], rhs=b2_sb[l][:],
                                     start=False, stop=True)
                    h_store(row0 + r, hp)

            # ---------------- phase B: layer 1 + chunked AllGather ----------
            def store_h1(rt, hp):
                hs = h1wp.tile([128, 256], H1DT, tag="h1")
                nc.scalar.activation(out=hs[:], in_=hp[:], func=AF.Relu)
                eng = nc.sync if rt % 2 == 0 else nc.scalar
                eng.dma_start(
                    out=h1loc[rt * 128:(rt + 1) * 128, :], in_=hs[:])

            NCHK = NTILE // 4
            ag_after = {}
            for cgi in range(NCHUNK):
                ag_after[-(-CH * (cgi + 1) // 512) - 1] = cgi
            for ch in range(NCHK):
                cnt_sb = work.tile([19, 512], f16, tag="cnt")
                nc.sync.dma_start(out=cnt_sb[:],
                                  in_=cnt19[:, ch * 512:(ch + 1) * 512])
                agg_sb = []
                for k in range(2):
                    ap_ = ps512.tile([128, 512], f32, space="PSUM",
                                     tag="ps512")
                    nc.tensor.matmul(
                        ap_[:], lhsT=tab1_sb[:, k * 128:(k + 1) * 128],
                        rhs=cnt_sb[:], start=True, stop=True)
                    asb = aggp.tile([128, 512], f16, tag="agg")
                    nc.vector.tensor_copy(out=asb[:], in_=ap_[:])
                    agg_sb.append(asb)
                mlp(0, agg_sb, ch * 4, store_h1)
                if ch in ag_after:
                    cgi = ag_after[ch]
                    nc.gpsimd.collective_compute(
                        "AllGather", OP.bypass,
                        replica_groups=[list(range(NCORE))],
                        ins=[h1loc[cgi * CH:(cgi + 1) * CH, :].opt()],
                        outs=[h1full[cgi][:].opt()])

            # ---------------- phase C: layer 2 ------------------------------
            h2_tiles = []
            pool_done = [0]  # blocks pooled so far

            def store_h2(rt, hp):
                hs = h2p.tile([128, 256], f16, tag="h2")
                nc.scalar.activation(out=hs[:], in_=hp[:], func=AF.Relu)
                h2_tiles.append(hs)

            def pool_block(b):
                pp_ = psp.tile([128, 256], f32, space="PSUM", tag="pp",
                               name="pp_")
                pps = [pp_[:, 0:128], pp_[:, 128:256]]
                ops = []
                for s in range(S):
                    nt = b * S + s
                    op_ = ohp.tile([128, 128], f16, tag="ohp")
                    nc.vector.tensor_scalar(
                        out=op_[:], in0=colidx_sb[:],
                        scalar1=indloc_sb[:, nt:nt + 1], scalar2=None,
                        op0=OP.is_equal)
                    ops.append(op_)
                for k in range(2):
                    for s in range(S):
                        nt = b * S + s
                        nc.tensor.matmul(
                            pps[k],
                            lhsT=h2_tiles[nt][:, k * 128:(k + 1) * 128],
                            rhs=ops[s][:], start=(s == 0), stop=(s == S - 1))
                    nc.vector.tensor_copy(
                        out=cat_sb[2 + k][:, b * 128:(b + 1) * 128],
                        in_=pps[k])

            agg2_box = [None]

            def gather_gb(gb):
                gout = []
                for cls in range(NCHUNK):
                    nidx = iw_len[gb][cls]
                    gtiles = nidx // 128
                    if gtiles == 0:
                        gout.append(None)
                        continue
                    gt = goutp.tile([128, meta["GMAX"], 256], H1DT,
                                    tag=f"g{cls}", name=f"gout{cls}")
                    nc.gpsimd.dma_gather(
                        out_ap=gt[:, 0:gtiles, :],
                        in_ap=h1full[cls][:, :],
                        idxs_ap=gidx_sb[:, iw_off[gb][cls]:
                                        iw_off[gb][cls] + nidx // 16],
                        num_idxs=nidx,
                        num_idxs_reg=nidx,
                        elem_size=256,
                    )
                    gout.append(gt)
                return gout

            def make_ohs(nt):
                ohs = []
                for cls in range(NCHUNK):
                    for j in range(Tmax[nt][cls]):
                        tg = tile_off[nt][cls] + j
                        oh_sb = ohp.tile([128, 128], f16, tag="oh")
                        eng = nc.sync if tg % 2 == 0 else nc.scalar
                        eng.dma_start(out=oh_sb[:], in_=oh_e[tg])
                        ohs.append(oh_sb)
                return ohs

            def scatter_nt_k(gb, gout, nt, gps, ntiles, ohs, k):
                ti = 0
                for cls in range(NCHUNK):
                    # group offset of this nt inside gout[cls]
                    goff = sum(Tmax[nt2][cls]
                               for nt2 in range(gb * GB_NT, nt))
                    for j in range(Tmax[nt][cls]):
                        oh_sb = ohs[ti]
                        ti += 1
                        nc.tensor.matmul(
                            gps[k],
                            lhsT=gout[cls][:, goff + j,
                                           k * 128:(k + 1) * 128],
                            rhs=oh_sb[:], start=False, stop=(ti == ntiles))

            def do_nt(gb, gout, nt):
                if nt % 4 == 0:
                    agg2_box[0] = [aggp.tile([128, 512], f16, tag="agg",
                                             name=f"agg2_{k}")
                                   for k in range(2)]
                agg2_sb = agg2_box[0]
                gp_ = psg.tile([128, 256], f32, space="PSUM", tag="gp",
                               name="gp_")
                gps = [gp_[:, 0:128], gp_[:, 128:256]]
                ntiles = sum(Tmax[nt][cls] for cls in range(NCHUNK))
                assert ntiles > 0
                selft = selfp.tile([128, 256], H1DT, tag="selft")
                seng = nc.sync if nt % 2 == 0 else nc.scalar
                seng.dma_start(out=selft[:],
                               in_=h1loc[nt * 128:(nt + 1) * 128, :])
                ohs = make_ohs(nt)
                for k in range(2):
                    nc.tensor.matmul(
                        gps[k], lhsT=ee2_sb[:, k * 128:(k + 1) * 128],
                        rhs=cnt10_sb[:, nt * 128:(nt + 1) * 128],
                        start=True, stop=False)
                    nc.tensor.matmul(
                        gps[k], lhsT=selft[:, k * 128:(k + 1) * 128],
                        rhs=ident[:], start=False, stop=False)
                    scatter_nt_k(gb, gout, nt, gps, ntiles, ohs, k)
                    nc.vector.tensor_copy(
                        out=agg2_sb[k][:, (nt % 4) * 128:(nt % 4 + 1) * 128],
                        in_=gps[k])
                if nt % 4 == 3:
                    ch = nt // 4
                    if DBG_TAPS:
                        for k in range(2):
                            nc.sync.dma_start(
                                out=dbg_agg2[k][:, ch * 512:(ch + 1) * 512],
                                in_=agg2_sb[k][:])
                    mlp(1, agg2_sb, ch * 4, store_h2)
                    while ((pool_done[0] + 1) * S - 1) // 4 <= ch:
                        pool_block(pool_done[0])
                        pool_done[0] += 1

            for gb in range(NGB):
                gout = gather_gb(gb)
                for nt in range(gb * GB_NT, (gb + 1) * GB_NT):
                    do_nt(gb, gout, nt)
            assert pool_done[0] == NBLK, pool_done

            # ---------------- phase D: BN1 -> proj -> BN2 -> out ------------
            if DBG_TAPS:
                for t in range(4):
                    nc.sync.dma_start(out=dbg_cat[t], in_=cat_sb[t][:])
                nc.sync.dma_start(out=dbg_h1[:, :], in_=h1loc[:, :])
            for t in range(2):
                tile_stats(cat_sb[2 + t], st1loc, 2, t)
            nc.gpsimd.collective_compute(
                "AllReduce", OP.add, replica_groups=[list(range(NCORE))],
                ins=[st1loc[:].opt()], outs=[st1glob[:].opt()])

            def scale_bias(g_sb, gam, bet):
                mu = work.tile([128, 1], f32, tag="mu")
                nc.vector.tensor_scalar_mul(mu[:], g_sb[:, 0:1], 1.0 / N)
                var = work.tile([128, 1], f32, tag="var")
                nc.vector.tensor_scalar_mul(var[:], g_sb[:, 1:2], 1.0 / N)
                musq = work.tile([128, 1], f32, tag="musq")
                nc.vector.tensor_tensor(out=musq[:], in0=mu[:], in1=mu[:],
                                        op=OP.mult)
                nc.vector.tensor_tensor(out=var[:], in0=var[:], in1=musq[:],
                                        op=OP.subtract)
                sd = work.tile([128, 1], f32, tag="sd")
                nc.scalar.activation(out=sd[:], in_=var[:], func=AF.Sqrt,
                                     bias=eps_sb[:, 0:1])
                rstd = work.tile([128, 1], f32, tag="rstd")
                nc.vector.reciprocal(rstd[:], sd[:])
                sc = work.tile([128, 1], f32, tag="sc")
                nc.vector.tensor_tensor(out=sc[:], in0=rstd[:], in1=gam[:],
                                        op=OP.mult)
                bi = work.tile([128, 1], f32, tag="bi")
                nc.vector.tensor_tensor(out=bi[:], in0=mu[:], in1=sc[:],
                                        op=OP.mult)
                nc.vector.tensor_tensor(out=bi[:], in0=bet[:], in1=bi[:],
                                        op=OP.subtract)
                return sc, bi

            bi_sb = []
            for t in range(4):
                g_sb = statp.tile([128, 2], f32, tag="gstat")
                nc.sync.dma_start(out=g_sb[:],
                                  in_=st1glob[t * 128:(t + 1) * 128, :])
                sc, bi = scale_bias(g_sb, bng_sb[t], bnb_sb[t])
                nc.vector.tensor_scalar(
                    out=cat_sb[t][:], in0=cat_sb[t][:], scalar1=sc[:, 0:1],
                    scalar2=bi[:, 0:1], op0=OP.mult, op1=OP.add)
                bi_sb.append(bi)

            out2_sb = [wide.tile([128, CPAD], f32, tag=f"o2_{c2}",
                                 name=f"o2sb{c2}") for c2 in range(2)]
            for w in range(CPAD // 512):
                for c2 in range(2):
                    pp = ps512.tile([128, 512], f32, space="PSUM",
                                    tag="ps512")
                    for k in range(4):
                        nc.tensor.matmul(
                            pp[:], lhsT=wp_sb[k][:, c2 * 128:(c2 + 1) * 128],
                            rhs=cat_sb[k][:, w * 512:(w + 1) * 512],
                            start=(k == 0), stop=(k == 3))
                    nc.vector.tensor_scalar(
                        out=out2_sb[c2][:, w * 512:(w + 1) * 512], in0=pp[:],
                        scalar1=bp_sb[c2][:, 0:1], scalar2=None, op0=OP.add)

            # v[c2] = proj of the all-pad (zero cat) column, for stats fixup
            v_sb = []
            for c2 in range(2):
                vp_ = ps512.tile([128, 512], f32, space="PSUM", tag="ps512")
                vp = vp_[:, 0:1]
                for k in range(4):
                    nc.tensor.matmul(
                        vp, lhsT=wp_sb[k][:, c2 * 128:(c2 + 1) * 128],
                        rhs=bi_sb[k][:], start=(k == 0), stop=(k == 3))
                v_ = work.tile([128, 1], f32, tag="v")
                nc.vector.tensor_tensor(out=v_[:], in0=vp,
                                        in1=bp_sb[c2][:], op=OP.add)
                v_sb.append(v_)

            for c2 in range(2):
                s_sb = statp.tile([128, 2], f32, tag="stat2")
                nc.vector.tensor_reduce(
                    out=s_sb[:, 0:1], in_=out2_sb[c2][:],
                    axis=mybir.AxisListType.X, op=OP.add)
                s1t = statp.tile([128, 1], f32, tag="s1u")
                for w in range(CPAD // 512):
                    nc.vector.tensor_tensor(
                        out=sqtmp[:], in0=out2_sb[c2][:, w * 512:(w + 1) * 512],
                        in1=out2_sb[c2][:, w * 512:(w + 1) * 512], op=OP.mult)
                    if w == 0:
                        nc.vector.tensor_reduce(
                            out=s_sb[:, 1:2], in_=sqtmp[:],
                            axis=mybir.AxisListType.X, op=OP.add)
                    else:
                        nc.vector.tensor_reduce(
                            out=s1t[:], in_=sqtmp[:],
                            axis=mybir.AxisListType.X, op=OP.add)
                        nc.vector.tensor_tensor(
                            out=s_sb[:, 1:2], in0=s_sb[:, 1:2], in1=s1t[:],
                            op=OP.add)
                # remove the NPAD fake columns (each equals v)
                vs = work.tile([128, 1], f32, tag="vs")
                nc.vector.tensor_scalar_mul(vs[:], v_sb[c2][:], float(NPAD))
                nc.vector.tensor_tensor(out=s_sb[:, 0:1], in0=s_sb[:, 0:1],
                                        in1=vs[:], op=OP.subtract)
                vs2 = work.tile([128, 1], f32, tag="vs2")
                nc.vector.tensor_tensor(out=vs2[:], in0=v_sb[c2][:],
                                        in1=vs[:], op=OP.mult)
                nc.vector.tensor_tensor(out=s_sb[:, 1:2], in0=s_sb[:, 1:2],
                                        in1=vs2[:], op=OP.subtract)
                nc.sync.dma_start(out=st2loc[c2 * 128:(c2 + 1) * 128, :],
                                  in_=s_sb[:])
            nc.gpsimd.collective_compute(
                "AllReduce", OP.add, replica_groups=[list(range(NCORE))],
                ins=[st2loc[:].opt()], outs=[st2glob[:].opt()])
            for c2 in range(2):
                g_sb = statp.tile([128, 2], f32, tag="gstat2")
                nc.sync.dma_start(out=g_sb[:],
                                  in_=st2glob[c2 * 128:(c2 + 1) * 128, :])
                sc, bi = scale_bias(g_sb, ng_sb[c2], nb_sb[c2])
                o16 = wide.tile([128, CPAD], f16, tag=f"o16_{c2}",
                                name=f"o16_{c2}")
                nc.vector.tensor_scalar(
                    out=o16[:], in0=out2_sb[c2][:],
                    scalar1=sc[:, 0:1], scalar2=bi[:, 0:1],
                    op0=OP.mult, op1=OP.add)
                eng = nc.sync if c2 == 0 else nc.scalar
                eng.dma_start(out=out[c2], in_=o16[:])

    nc.compile()
    return nc


_CACHE = {}


def kernel(**inputs):
    from concourse.bass_utils import run_bass_kernel_spmd

    per_core, meta, colmap = _preprocess(inputs)
    wm = _weight_maps(inputs)

    key = (meta["ROWS"], meta["T_E"],
           tuple(tuple(r) for r in meta["Tmax"]))
    if key not in _CACHE:
        _CACHE[key] = _build(meta)
    nc = _CACHE[key]

    in_maps = []
    for c in range(NCORE):
        m = dict(per_core[c])
        m.update(wm)
        in_maps.append(m)

    trace = bool(int(os.environ.get("KERNEL_TRACE", "0")))
    res = run_bass_kernel_spmd(nc, in_maps, list(range(NCORE)), trace=trace)
    kernel.last_results = res

    out_np = np.empty((N, D), np.float32)
    for c in range(NCORE):
        arr = res.results[c]["out"].astype(np.float32)  # [2, 128, CPAD]
        cols = np.flatnonzero(colmap[c] >= 0)
        ids = colmap[c][cols]
        out_np[ids, :128] = arr[0][:, cols].T
        out_np[ids, 128:] = arr[1][:, cols].T
    return out_np
